# revision 1
# baseline (speedup 1.0000x reference)
import os
import sys

os.environ.setdefault("JAX_PLATFORMS", "")
sys.path.insert(0, "/opt/trn_rl_repo")

import numpy as np
import ml_dtypes

BF16 = ml_dtypes.bfloat16
INV = np.float32(1.0 / np.sqrt(1.0 + 1e-5))
GROUPS = [[0, 1, 2, 3], [4, 5, 6, 7]]
NLOC = 1024
CHUNKS = (0, 512)

_CACHE = {}


def _hilo(a):
    a = np.ascontiguousarray(np.asarray(a, dtype=np.float32))
    hi = a.astype(BF16)
    lo = (a - hi.astype(np.float32)).astype(BF16)
    return hi, lo


def _blockP(a):
    a = np.ascontiguousarray(a)
    R, C = a.shape
    assert R % 128 == 0, (R, C)
    nb = R // 128
    return np.ascontiguousarray(a.reshape(nb, 128, C).transpose(1, 0, 2).reshape(128, nb * C))


# name -> (shape, "bf"/"f32")
SPECS = {
    "xT_hi": ((3, 1024), "bf"), "xT_lo": ((3, 1024), "bf"),
    "c1T_hi": ((3, 64), "bf"), "c1T_lo": ((3, 64), "bf"),
    "c2T_hi": ((64, 128), "bf"), "c2T_lo": ((64, 128), "bf"),
    "c3T_hi": ((128, 256), "bf"), "c3T_lo": ((128, 256), "bf"),
    "p1T_hi": ((128, 512), "bf"), "p1T_lo": ((128, 512), "bf"),
    "p2T_hi": ((128, 512), "bf"), "p2T_lo": ((128, 512), "bf"),
    "fb1": ((128, 1), "f32"), "fb2": ((128, 1), "f32"), "fb3": ((128, 2), "f32"),
    "fbp1": ((128, 2), "f32"), "fbp2": ((128, 2), "f32"),
    "qkT0_hi": ((128, 256), "bf"), "qkT0_lo": ((128, 256), "bf"),
    "qkT1_hi": ((128, 256), "bf"), "qkT1_lo": ((128, 256), "bf"),
    "vwT0_hi": ((128, 512), "bf"), "vwT0_lo": ((128, 512), "bf"),
    "vwT1_hi": ((128, 512), "bf"), "vwT1_lo": ((128, 512), "bf"),
    "twT0_hi": ((128, 512), "bf"), "twT0_lo": ((128, 512), "bf"),
    "twT1_hi": ((128, 512), "bf"), "twT1_lo": ((128, 512), "bf"),
    "vbb0": ((128, 256), "f32"), "vbb1": ((128, 256), "f32"),
    "tbf0": ((128, 2), "f32"), "tbf1": ((128, 2), "f32"),
    "cfT_hi": ((128, 4096), "bf"), "cfT_lo": ((128, 4096), "bf"),
    "cfb": ((128, 4), "f32"),
    "s1fT_hi": ((128, 2048), "bf"), "s1fT_lo": ((128, 2048), "bf"),
    "s1gT_hi": ((128, 2048), "bf"), "s1gT_lo": ((128, 2048), "bf"),
    "s1b": ((128, 4), "f32"),
    "s2T_hi": ((128, 1024), "bf"), "s2T_lo": ((128, 1024), "bf"),
    "s2b": ((128, 2), "f32"),
    "s3T_hi": ((128, 100), "bf"), "s3T_lo": ((128, 100), "bf"),
}


DBG_SPECS = {
    "d_front": ((128, 2048), "f32"),
    "d_xq0": ((128, 1024), "bf"),
    "d_xvt0": ((128, 2048), "bf"),
    "d_xqf": ((128, 4096), "bf"),
    "d_rsg": ((128, 32), "f32"),
    "d_P0": ((128, 1024), "bf"),
    "d_cs": ((1, 1024), "f32"),
    "d_icb": ((128, 1024), "f32"),
    "d_xrh": ((128, 2048), "bf"),
    "d_A": ((128, 2048), "f32"),
    "d_xm1": ((128, 2048), "f32"),
    "d_xm4": ((128, 2048), "f32"),
    "d_faceh": ((128, 4096), "bf"),
    "d_gml": ((128, 4), "f32"),
    "d_gmg": ((128, 4), "f32"),
    "d_gb": ((128, 4), "f32"),
    "d_h2h": ((128, 4096), "bf"),
    "d_h3h": ((128, 2048), "bf"),
}


def _build(debug=False, reps=1):
    from concourse import tile, bacc, mybir

    dt = mybir.dt
    AF = mybir.ActivationFunctionType
    AX = mybir.AxisListType
    ALU = mybir.AluOpType
    bf, f32 = dt.bfloat16, dt.float32

    nc = bacc.Bacc("TRN2", target_bir_lowering=False, debug=False, num_devices=8)

    D = {}
    for nm, (sh, dn) in SPECS.items():
        D[nm] = nc.dram_tensor(nm, list(sh), bf if dn == "bf" else f32, kind="ExternalInput")
    out_d = nc.dram_tensor("out", [50, 1024], f32, kind="ExternalOutput")
    DBG = {}
    if debug:
        for nm, (sh, dn) in DBG_SPECS.items():
            DBG[nm] = nc.dram_tensor(nm, list(sh), bf if dn == "bf" else f32,
                                     kind="ExternalOutput")

    def tap(nm, t):
        if debug and nm in DBG:
            nc.gpsimd.dma_start(DBG[nm][:], t[:])

    with tile.TileContext(nc) as tc:
        with (
            tc.tile_pool(name="pers", bufs=1) as pers,
            tc.tile_pool(name="dramp", bufs=1, space="DRAM") as dramp,
        ):
            def sload(pool, nm):
                sh, dn = SPECS[nm]
                t = pool.tile(list(sh), bf if dn == "bf" else f32, tag=nm, name=nm + "_sb")
                nc.gpsimd.dma_start(t[:], D[nm][:])
                return t

            S = {}
            for nm in ("qkT0_hi", "qkT0_lo", "qkT1_hi", "qkT1_lo",
                       "vwT0_hi", "vwT0_lo", "vwT1_hi", "vwT1_lo",
                       "twT0_hi", "twT0_lo", "twT1_hi", "twT1_lo",
                       "vbb0", "vbb1", "tbf0", "tbf1"):
                S[nm] = sload(pers, nm)

            xm = pers.tile([128, 2048], f32, tag="xm")
            slot_h = [pers.tile([128, 2048], bf, tag=f"sh{s}", name=f"sh{s}") for s in range(5)]
            slot_l = [pers.tile([128, 2048], bf, tag=f"sl{s}", name=f"sl{s}") for s in range(5)]
            ones1 = pers.tile([1, 128], f32, tag="ones1")
            nc.gpsimd.memset(ones1[:], 1.0)

            ag_in = dramp.tile([128, 3072], bf, tag="ag_in")
            ag_out = dramp.tile([512, 3072], bf, tag="ag_out")
            ar_in = dramp.tile([128, 32], f32, tag="ar_in")
            ar_out = dramp.tile([128, 32], f32, tag="ar_out")
            gm_in = dramp.tile([128, 4], f32, tag="gm_in")
            gm_out = dramp.tile([128, 4], f32, tag="gm_out")

            def emit_pipeline():
                # ---------------- front chain ----------------
                with (
                    tc.tile_pool(name="fp", bufs=1) as fp,
                    tc.tile_pool(name="fps", bufs=1, space="PSUM") as fps,
                ):
                    for nm in ("xT_hi", "xT_lo", "c1T_hi", "c1T_lo", "c2T_hi", "c2T_lo",
                               "c3T_hi", "c3T_lo", "p1T_hi", "p1T_lo", "p2T_hi", "p2T_lo",
                               "fb1", "fb2", "fb3", "fbp1", "fbp2"):
                        S[nm] = sload(fp, nm)

                    chain = [
                        ("c1T", 1, 64, "fb1"),
                        ("c2T", 1, 128, "fb2"),
                        ("c3T", 1, 256, "fb3"),
                        ("p1T", 2, 256, "fbp1"),
                        ("p2T", 2, 256, "fbp2"),
                    ]
                    cur_hi, cur_lo = S["xT_hi"], S["xT_lo"]
                    for li, (wn, kb, Cout, bn) in enumerate(chain):
                        whi, wlo, bt = S[wn + "_hi"], S[wn + "_lo"], S[bn]
                        nob = (Cout + 127) // 128
                        P_out = min(Cout, 128)
                        last = li == len(chain) - 1
                        if last:
                            of = xm
                        else:
                            of = fp.tile([P_out, nob * 1024], f32, tag=f"hf{li}", name=f"hf{li}")
                        for ob in range(nob):
                            Mob = min(128, Cout - 128 * ob)
                            for c0 in CHUNKS:
                                ps = fps.tile([Mob, 512], f32, tag="fps_t", bufs=4, name="fps_t")
                                n, tot = 0, kb * 3
                                for kbi in range(kb):
                                    lh = whi[:, kbi * Cout + 128 * ob: kbi * Cout + 128 * ob + Mob]
                                    ll = wlo[:, kbi * Cout + 128 * ob: kbi * Cout + 128 * ob + Mob]
                                    rh = cur_hi[:, kbi * 1024 + c0: kbi * 1024 + c0 + 512]
                                    rl = cur_lo[:, kbi * 1024 + c0: kbi * 1024 + c0 + 512]
                                    for lt, rt in ((lh, rh), (lh, rl), (ll, rh)):
                                        nc.tensor.matmul(ps[:], lt, rt, start=(n == 0), stop=(n == tot - 1))
                                        n += 1
                                nc.scalar.activation(
                                    of[0:Mob, ob * 1024 + c0: ob * 1024 + c0 + 512], ps[:],
                                    AF.Relu, bias=bt[0:Mob, ob: ob + 1],
                                )
                        if last:
                            nh, nl = slot_h[0], slot_l[0]
                        else:
                            nh = fp.tile([P_out, nob * 1024], bf, tag=f"hh{li}", name=f"hh{li}")
                            nl = fp.tile([P_out, nob * 1024], bf, tag=f"hl{li}", name=f"hl{li}")
                        hscr = fp.tile([P_out, nob * 1024], f32, tag="hscr", name="hscr")
                        W = nob * 1024
                        nc.vector.tensor_copy(nh[0:P_out, 0:W], of[0:P_out, 0:W])
                        nc.vector.tensor_copy(hscr[0:P_out, 0:W], nh[0:P_out, 0:W])
                        nc.vector.tensor_sub(nl[0:P_out, 0:W], of[0:P_out, 0:W], hscr[0:P_out, 0:W])
                        cur_hi, cur_lo = nh, nl
                tap("d_front", xm)

                # ---------------- 4 SA layers ----------------
                with tc.tile_pool(name="sap", bufs=1) as sp:
                    for L in range(4):
                        v = 0 if L == 0 else 1
                        ih, il = slot_h[L], slot_l[L]
                        oh, ol = slot_h[L + 1], slot_l[L + 1]
                        qh = S[f"qkT{v}_hi"]
                        vh = S[f"vwT{v}_hi"]
                        th = S[f"twT{v}_hi"]
                        vbb, tbt = S[f"vbb{v}"], S[f"tbf{v}"]

                        xq_loc = sp.tile([128, 1024], bf, tag="xq_loc", name="xq_loc")
                        xvt_loc = sp.tile([128, 2048], bf, tag="xvt_loc", name="xvt_loc")
                        xq_full = sp.tile([128, 4096], bf, tag="xq_full", name="xq_full")
                        xvt_full = sp.tile([128, 8192], bf, tag="xvt_full", name="xvt_full")
                        P_sb = sp.tile([128, 32 * 1024], bf, tag="P_sb", name="P_sb")
                        rs_sb = sp.tile([128, 32], f32, tag="rs_sb", name="rs_sb")
                        rs_g = sp.tile([128, 32], f32, tag="rs_g", name="rs_g")
                        r_f = sp.tile([128, 32], f32, tag="r_f", name="r_f")
                        r_b = sp.tile([128, 32], bf, tag="r_b", name="r_b")
                        A_sb = sp.tile([128, 2048], f32, tag="A_sb", name="A_sb")
                        xr_hi = sp.tile([128, 2048], bf, tag="xr_hi", name="xr_hi")
                        icb = sp.tile([128, 1024], f32, tag="icb", name="icb")
                        cs_row = sp.tile([1, 1024], f32, tag="cs_row", name="cs_row")
                        ic_row = sp.tile([1, 1024], f32, tag="ic_row", name="ic_row")

                        # phase 0: local xq and xv^T (plain bf16)
                        with tc.tile_pool(name="ps0", bufs=1, space="PSUM") as ps0:
                            for c0 in CHUNKS:
                                qps = ps0.tile([128, 512], f32, tag="qps", bufs=2, name="qps")
                                for kbi in range(2):
                                    nc.tensor.matmul(
                                        qps[:], qh[:, kbi * 128: kbi * 128 + 128],
                                        ih[:, kbi * 1024 + c0: kbi * 1024 + c0 + 512],
                                        start=(kbi == 0), stop=(kbi == 1))
                                nc.scalar.activation(xq_loc[:, c0: c0 + 512], qps[:], AF.Copy)
                            for nb in range(8):
                                vps = ps0.tile([128, 256], f32, tag="vps", bufs=2, name="vps")
                                for kbi in range(2):
                                    nc.tensor.matmul(
                                        vps[:], ih[:, kbi * 1024 + nb * 128: kbi * 1024 + nb * 128 + 128],
                                        vh[:, kbi * 256: kbi * 256 + 256],
                                        start=(kbi == 0), stop=(kbi == 1))
                                nc.vector.tensor_add(xvt_loc[:, nb * 256: nb * 256 + 256], vps[:], vbb[:])

                        if L == 0:
                            tap("d_xq0", xq_loc)
                            tap("d_xvt0", xvt_loc)

                        # phase 1: gather xq + xv^T
                        nc.gpsimd.dma_start(ag_in[:, 0:1024], xq_loc[:])
                        nc.gpsimd.dma_start(ag_in[:, 1024:3072], xvt_loc[:])
                        nc.gpsimd.collective_compute(
                            "AllGather", ALU.bypass, replica_groups=GROUPS,
                            ins=[ag_in.opt()], outs=[ag_out.opt()],
                        )
                        for k in range(4):
                            nc.gpsimd.dma_start(xq_full[:, k * 1024: (k + 1) * 1024],
                                                ag_out[k * 128: (k + 1) * 128, 0:1024])
                            nc.gpsimd.dma_start(xvt_full[:, k * 2048: (k + 1) * 2048],
                                                ag_out[k * 128: (k + 1) * 128, 1024:3072])

                        # phase 2: energy + exp(P) + rowsum partials; A = tw@x off critical path
                        with tc.tile_pool(name="ps2", bufs=1, space="PSUM") as ps2:
                            for g in range(32):
                                eps = ps2.tile([128, 1024], f32, tag="eps", bufs=2, name="eps")
                                lhs = xq_full[:, g * 128: (g + 1) * 128]
                                nc.tensor.matmul(eps[:, 0:512], lhs, xq_loc[:, 0:512], start=True, stop=True)
                                nc.tensor.matmul(eps[:, 512:1024], lhs, xq_loc[:, 512:1024], start=True, stop=True)
                                nc.scalar.activation(P_sb[:, g * 1024: (g + 1) * 1024], eps[:],
                                                     AF.Exp, accum_out=rs_sb[:, g: g + 1])
                            for ob in range(2):
                                for c0 in CHUNKS:
                                    aps = ps2.tile([128, 512], f32, tag="aps", bufs=2, name="aps")
                                    for kbi in range(2):
                                        nc.tensor.matmul(
                                            aps[:], th[:, kbi * 256 + 128 * ob: kbi * 256 + 128 * ob + 128],
                                            ih[:, kbi * 1024 + c0: kbi * 1024 + c0 + 512],
                                            start=(kbi == 0), stop=(kbi == 1))
                                    nc.scalar.activation(A_sb[:, ob * 1024 + c0: ob * 1024 + c0 + 512],
                                                         aps[:], AF.Copy)

                        if L == 0:
                            tap("d_xqf", xq_full)
                            tap("d_A", A_sb)
                            if debug:
                                nc.gpsimd.dma_start(DBG["d_P0"][:], P_sb[:, 0:1024])

                        # phase 3: rowsum allreduce, scale, colsum, apply
                        nc.gpsimd.dma_start(ar_in[:], rs_sb[:])
                        nc.gpsimd.collective_compute(
                            "AllReduce", ALU.add, replica_groups=GROUPS,
                            ins=[ar_in.opt()], outs=[ar_out.opt()],
                        )
                        nc.gpsimd.dma_start(rs_g[:], ar_out[:])
                        nc.vector.reciprocal(r_f[:], rs_g[:])
                        nc.vector.tensor_copy(r_b[:], r_f[:])
                        for g in range(32):
                            nc.vector.tensor_scalar_mul(
                                xvt_full[:, g * 256: (g + 1) * 256],
                                xvt_full[:, g * 256: (g + 1) * 256], r_f[:, g: g + 1])

                        with tc.tile_pool(name="ps3", bufs=1, space="PSUM") as ps3:
                            for ci, c0 in enumerate(CHUNKS):
                                csp = ps3.tile([1, 512], f32, tag=f"csp{ci}", name=f"csp{ci}")
                                for g in range(32):
                                    nc.tensor.matmul(csp[:], r_b[:, g: g + 1],
                                                     P_sb[:, g * 1024 + c0: g * 1024 + c0 + 512],
                                                     start=(g == 0), stop=(g == 31))
                                nc.vector.tensor_scalar_add(cs_row[:, c0: c0 + 512], csp[:], 1e-9)
                            nc.vector.reciprocal(ic_row[:], cs_row[:])
                            for c0 in CHUNKS:
                                ibp = ps3.tile([128, 512], f32, tag="ibp", bufs=2, name="ibp")
                                nc.tensor.matmul(ibp[:], ones1[:], ic_row[:, c0: c0 + 512], start=True, stop=True)
                                nc.scalar.activation(icb[:, c0: c0 + 512], ibp[:], AF.Copy)
                            for cb in range(2):
                                for c0 in CHUNKS:
                                    xrp = ps3.tile([128, 512], f32, tag="xrp", bufs=2, name="xrp")
                                    for g in range(32):
                                        nc.tensor.matmul(
                                            xrp[:],
                                            xvt_full[:, g * 256 + cb * 128: g * 256 + cb * 128 + 128],
                                            P_sb[:, g * 1024 + c0: g * 1024 + c0 + 512],
                                            start=(g == 0), stop=(g == 31))
                                    sl = slice(cb * 1024 + c0, cb * 1024 + c0 + 512)
                                    nc.scalar.activation(xr_hi[:, sl], xrp[:], AF.Copy)

                        if L == 0:
                            tap("d_rsg", rs_g)
                            tap("d_cs", cs_row)
                            tap("d_icb", icb)
                            tap("d_xrh", xr_hi)

                        # phase 4: B = tw@xr, y = relu(A - B*icb + tbf), resid add, splits
                        with tc.tile_pool(name="ps4", bufs=1, space="PSUM") as ps4:
                            for ob in range(2):
                                for c0 in CHUNKS:
                                    bps = ps4.tile([128, 512], f32, tag="bps", bufs=2, name="bps")
                                    for kbi in range(2):
                                        nc.tensor.matmul(
                                            bps[:], th[:, kbi * 256 + 128 * ob: kbi * 256 + 128 * ob + 128],
                                            xr_hi[:, kbi * 1024 + c0: kbi * 1024 + c0 + 512],
                                            start=(kbi == 0), stop=(kbi == 1))
                                    osl = slice(ob * 1024 + c0, ob * 1024 + c0 + 512)
                                    tmp = sp.tile([128, 512], f32, tag="scr", bufs=2, name="tmp")
                                    nc.vector.tensor_mul(tmp[:], bps[:], icb[:, c0: c0 + 512])
                                    nc.vector.tensor_sub(A_sb[:, osl], A_sb[:, osl], tmp[:])
                                    yv = sp.tile([128, 512], f32, tag="scr", bufs=2, name="yv")
                                    nc.scalar.activation(yv[:], A_sb[:, osl], AF.Relu, bias=tbt[:, ob: ob + 1])
                                    nc.vector.tensor_add(xm[:, osl], xm[:, osl], yv[:])
                        for c0 in (0, 512, 1024, 1536):
                            sscr = sp.tile([128, 512], f32, tag="scr", bufs=2, name="sscr")
                            nc.vector.tensor_copy(oh[:, c0: c0 + 512], xm[:, c0: c0 + 512])
                            nc.vector.tensor_copy(sscr[:], oh[:, c0: c0 + 512])
                            nc.vector.tensor_sub(ol[:, c0: c0 + 512], xm[:, c0: c0 + 512], sscr[:])
                        if L == 0:
                            tap("d_xm1", xm)
                        if L == 3:
                            tap("d_xm4", xm)

                # ---------------- back end ----------------
                with tc.tile_pool(name="bp", bufs=1) as bp:
                    for nm in ("cfT_hi", "cfT_lo", "cfb", "s1fT_hi", "s1fT_lo",
                               "s1gT_hi", "s1gT_lo", "s1b", "s2T_hi", "s2T_lo",
                               "s2b", "s3T_hi", "s3T_lo"):
                        S[nm] = sload(bp, nm)

                    face_hi = bp.tile([128, 4096], bf, tag="face_hi", name="face_hi")
                    face_lo = bp.tile([128, 4096], bf, tag="face_lo", name="face_lo")
                    gml = bp.tile([128, 4], f32, tag="gml", name="gml")

                    with tc.tile_pool(name="psA", bufs=1, space="PSUM") as psA:
                        for ob in range(4):
                            fscr = bp.tile([128, 1024], f32, tag="fscr", bufs=2, name="fscr")
                            for c0 in CHUNKS:
                                fpt = psA.tile([128, 512], f32, tag="fpsb", bufs=4, name="fpt")
                                n, tot = 0, 24
                                for sk in range(8):
                                    s, cb = 1 + sk // 2, sk % 2
                                    rh = slot_h[s][:, cb * 1024 + c0: cb * 1024 + c0 + 512]
                                    rl = slot_l[s][:, cb * 1024 + c0: cb * 1024 + c0 + 512]
                                    lh = S["cfT_hi"][:, sk * 512 + 128 * ob: sk * 512 + 128 * ob + 128]
                                    ll = S["cfT_lo"][:, sk * 512 + 128 * ob: sk * 512 + 128 * ob + 128]
                                    for lt, rt in ((lh, rh), (lh, rl), (ll, rh)):
                                        nc.tensor.matmul(fpt[:], lt, rt, start=(n == 0), stop=(n == tot - 1))
                                        n += 1
                                nc.scalar.activation(fscr[:, c0: c0 + 512], fpt[:], AF.Prelu,
                                                     bias=S["cfb"][:, ob: ob + 1], alpha=0.2)
                            nc.vector.tensor_reduce(gml[:, ob: ob + 1], fscr[:], axis=AX.X, op=ALU.max)
                            sl = slice(ob * 1024, (ob + 1) * 1024)
                            f2 = bp.tile([128, 1024], f32, tag="f2scr", bufs=2, name="f2")
                            nc.vector.tensor_copy(face_hi[:, sl], fscr[:])
                            nc.vector.tensor_copy(f2[:], face_hi[:, sl])
                            nc.vector.tensor_sub(face_lo[:, sl], fscr[:], f2[:])

                        nc.gpsimd.dma_start(gm_in[:], gml[:])
                        nc.gpsimd.collective_compute(
                            "AllReduce", ALU.max, replica_groups=GROUPS,
                            ins=[gm_in.opt()], outs=[gm_out.opt()],
                        )
                        gmg = bp.tile([128, 4], f32, tag="gmg", name="gmg")
                        nc.gpsimd.dma_start(gmg[:], gm_out[:])
                        gmh = bp.tile([128, 4], bf, tag="gmh", name="gmh")
                        gmlo = bp.tile([128, 4], bf, tag="gmlo", name="gmlo")
                        gms = bp.tile([128, 4], f32, tag="gms", name="gms")
                        nc.vector.tensor_copy(gmh[:], gmg[:])
                        nc.vector.tensor_copy(gms[:], gmh[:])
                        nc.vector.tensor_sub(gmlo[:], gmg[:], gms[:])

                        gb = bp.tile([128, 4], f32, tag="gb", name="gb")
                        for ob in range(4):
                            gvp = psA.tile([128, 1], f32, tag="gvp", bufs=2, name="gvp")
                            n, tot = 0, 12
                            for kbi in range(4):
                                lh = S["s1gT_hi"][:, kbi * 512 + 128 * ob: kbi * 512 + 128 * ob + 128]
                                ll = S["s1gT_lo"][:, kbi * 512 + 128 * ob: kbi * 512 + 128 * ob + 128]
                                rh = gmh[:, kbi: kbi + 1]
                                rl = gmlo[:, kbi: kbi + 1]
                                for lt, rt in ((lh, rh), (lh, rl), (ll, rh)):
                                    nc.tensor.matmul(gvp[:], lt, rt, start=(n == 0), stop=(n == tot - 1))
                                    n += 1
                            nc.vector.tensor_add(gb[:, ob: ob + 1], gvp[:], S["s1b"][:, ob: ob + 1])
                        tap("d_faceh", face_hi)
                        tap("d_gml", gml)
                        tap("d_gmg", gmg)
                        tap("d_gb", gb)

                    h2h = bp.tile([128, 4096], bf, tag="h2h", name="h2h")
                    h2l = bp.tile([128, 4096], bf, tag="h2l", name="h2l")
                    h3h = bp.tile([128, 2048], bf, tag="h3h", name="h3h")
                    h3l = bp.tile([128, 2048], bf, tag="h3l", name="h3l")
                    outsb = bp.tile([50, 1024], f32, tag="outsb", name="outsb")

                    with tc.tile_pool(name="psB", bufs=1, space="PSUM") as psB:
                        for ob in range(4):
                            uscr = bp.tile([128, 1024], f32, tag="fscr", bufs=2, name="uscr")
                            for c0 in CHUNKS:
                                sp1 = psB.tile([128, 512], f32, tag="sp1", bufs=2, name="sp1")
                                n, tot = 0, 12
                                for kbi in range(4):
                                    lh = S["s1fT_hi"][:, kbi * 512 + 128 * ob: kbi * 512 + 128 * ob + 128]
                                    ll = S["s1fT_lo"][:, kbi * 512 + 128 * ob: kbi * 512 + 128 * ob + 128]
                                    rh = face_hi[:, kbi * 1024 + c0: kbi * 1024 + c0 + 512]
                                    rl = face_lo[:, kbi * 1024 + c0: kbi * 1024 + c0 + 512]
                                    for lt, rt in ((lh, rh), (lh, rl), (ll, rh)):
                                        nc.tensor.matmul(sp1[:], lt, rt, start=(n == 0), stop=(n == tot - 1))
                                        n += 1
                                nc.scalar.activation(uscr[:, c0: c0 + 512], sp1[:], AF.Prelu,
                                                     bias=gb[:, ob: ob + 1], alpha=0.2)
                            sl = slice(ob * 1024, (ob + 1) * 1024)
                            f2 = bp.tile([128, 1024], f32, tag="f2scr", bufs=2, name="f2b")
                            nc.vector.tensor_copy(h2h[:, sl], uscr[:])
                            nc.vector.tensor_copy(f2[:], h2h[:, sl])
                            nc.vector.tensor_sub(h2l[:, sl], uscr[:], f2[:])

                        for ob in range(2):
                            u2 = bp.tile([128, 1024], f32, tag="fscr", bufs=2, name="u2")
                            for c0 in CHUNKS:
                                sp2 = psB.tile([128, 512], f32, tag="sp2", bufs=2, name="sp2")
                                n, tot = 0, 12
                                for kbi in range(4):
                                    lh = S["s2T_hi"][:, kbi * 256 + 128 * ob: kbi * 256 + 128 * ob + 128]
                                    ll = S["s2T_lo"][:, kbi * 256 + 128 * ob: kbi * 256 + 128 * ob + 128]
                                    rh = h2h[:, kbi * 1024 + c0: kbi * 1024 + c0 + 512]
                                    rl = h2l[:, kbi * 1024 + c0: kbi * 1024 + c0 + 512]
                                    for lt, rt in ((lh, rh), (lh, rl), (ll, rh)):
                                        nc.tensor.matmul(sp2[:], lt, rt, start=(n == 0), stop=(n == tot - 1))
                                        n += 1
                                nc.scalar.activation(u2[:, c0: c0 + 512], sp2[:], AF.Prelu,
                                                     bias=S["s2b"][:, ob: ob + 1], alpha=0.2)
                            sl = slice(ob * 1024, (ob + 1) * 1024)
                            f2 = bp.tile([128, 1024], f32, tag="f2scr", bufs=2, name="f2c")
                            nc.vector.tensor_copy(h3h[:, sl], u2[:])
                            nc.vector.tensor_copy(f2[:], h3h[:, sl])
                            nc.vector.tensor_sub(h3l[:, sl], u2[:], f2[:])

                        for c0 in CHUNKS:
                            sp3 = psB.tile([50, 512], f32, tag="sp3", bufs=2, name="sp3")
                            n, tot = 0, 6
                            for kbi in range(2):
                                lh = S["s3T_hi"][:, kbi * 50: kbi * 50 + 50]
                                ll = S["s3T_lo"][:, kbi * 50: kbi * 50 + 50]
                                rh = h3h[:, kbi * 1024 + c0: kbi * 1024 + c0 + 512]
                                rl = h3l[:, kbi * 1024 + c0: kbi * 1024 + c0 + 512]
                                for lt, rt in ((lh, rh), (lh, rl), (ll, rh)):
                                    nc.tensor.matmul(sp3[:], lt, rt, start=(n == 0), stop=(n == tot - 1))
                                    n += 1
                            nc.scalar.activation(outsb[:, c0: c0 + 512], sp3[:], AF.Copy)

                    tap("d_h2h", h2h)
                    tap("d_h3h", h3h)
                    nc.gpsimd.dma_start(out_d[:], outsb[:])

            for _ in range(reps):
                emit_pipeline()

    nc.compile()
    return nc


def _prep_shared(inputs):
    g = lambda k: np.asarray(inputs[k], np.float32)
    out = {}

    def fold(wn, gn, bn):
        return g(wn) * (INV * g(gn))[:, None], g(bn)

    def emit(nm, wf):
        wT = np.ascontiguousarray(wf.T)
        if wT.shape[0] > 128:
            wT = _blockP(wT)
        hi, lo = _hilo(wT)
        out[nm + "_hi"], out[nm + "_lo"] = hi, lo

    w1, b1 = fold("conv1_w", "bn1_g", "bn1_b")
    w2, b2 = fold("conv2_w", "bn2_g", "bn2_b")
    w3, b3 = fold("conv3_w", "bn3_g", "bn3_b")
    wp1, bp1 = fold("pt1_w", "pt1_g", "pt1_b")
    wp2, bp2 = fold("pt2_w", "pt2_g", "pt2_b")
    emit("c1T", w1)
    emit("c2T", w2)
    emit("c3T", w3)
    emit("p1T", wp1)
    emit("p2T", wp2)
    fb1 = np.zeros((128, 1), np.float32)
    fb1[:64, 0] = b1
    out["fb1"] = fb1
    out["fb2"] = np.ascontiguousarray(b2[:, None])
    out["fb3"] = _blockP(b3[:, None]).astype(np.float32)
    out["fbp1"] = _blockP(bp1[:, None]).astype(np.float32)
    out["fbp2"] = _blockP(bp2[:, None]).astype(np.float32)

    for v, p in ((0, "sa1"), (1, "sa2")):
        emit(f"qkT{v}", g(p + "_qk"))
        emit(f"vwT{v}", g(p + "_vw"))
        sg, sb2 = g(p + "_g"), g(p + "_b")
        twf = g(p + "_tw") * (INV * sg)[:, None]
        emit(f"twT{v}", twf)
        out[f"vbb{v}"] = np.ascontiguousarray(
            np.broadcast_to(g(p + "_vb")[None, :], (128, 256))).astype(np.float32)
        tbfv = g(p + "_tb") * (INV * sg) + sb2
        out[f"tbf{v}"] = _blockP(tbfv[:, None]).astype(np.float32)

    cfw, cfb_ = fold("cf_w", "cf_g", "cf_b")
    emit("cfT", cfw)
    out["cfb"] = _blockP(cfb_[:, None]).astype(np.float32)
    s1w, s1b_ = fold("s1_w", "s1_g", "s1_b")
    emit("s1fT", s1w[:, :512])
    emit("s1gT", s1w[:, 512:])
    out["s1b"] = _blockP(s1b_[:, None]).astype(np.float32)
    s2w, s2b_ = fold("s2_w", "s2_g", "s2_b")
    emit("s2T", s2w)
    out["s2b"] = _blockP(s2b_[:, None]).astype(np.float32)
    emit("s3T", g("s3_w"))

    for nm, (sh, dn) in SPECS.items():
        if nm.startswith("xT"):
            continue
        a = out[nm]
        assert tuple(a.shape) == sh, (nm, a.shape, sh)
        assert (a.dtype == BF16) == (dn == "bf"), (nm, a.dtype)
    return out


def _get_nc(debug=False, reps=1):
    key = ("nc_dbg" if debug else "nc") + str(reps)
    if key not in _CACHE:
        _CACHE[key] = _build(debug, reps)
    return _CACHE[key]


def _in_maps(inputs):
    base = _prep_shared(inputs)
    x = np.asarray(inputs["x"], np.float32)
    maps = []
    for c in range(8):
        b, j = c // 4, c % 4
        xT = np.ascontiguousarray(x[b, 1024 * j: 1024 * (j + 1), :].T)
        hi, lo = _hilo(xT)
        m = dict(base)
        m["xT_hi"], m["xT_lo"] = hi, lo
        maps.append(m)
    return maps


def _assemble(res):
    full = np.empty((2, 4096, 50), np.float32)
    for c in range(8):
        b, j = c // 4, c % 4
        full[b, 1024 * j: 1024 * (j + 1), :] = np.asarray(res.results[c]["out"], np.float32).T
    return full


def kernel(**inputs):
    from concourse.bass_utils import run_bass_kernel_spmd
    nc = _get_nc()
    res = run_bass_kernel_spmd(nc, _in_maps(inputs), core_ids=list(range(8)))
    return _assemble(res)


def run_traced(inputs, trace_cores=None):
    from concourse.bass_utils import run_bass_kernel_spmd
    nc = _get_nc()
    res = run_bass_kernel_spmd(
        nc, _in_maps(inputs), core_ids=list(range(8)),
        trace=True, trace_cores=trace_cores or [0],
    )
    return _assemble(res), res


def run_debug(inputs):
    from concourse.bass_utils import run_bass_kernel_spmd
    nc = _get_nc(debug=True)
    res = run_bass_kernel_spmd(nc, _in_maps(inputs), core_ids=list(range(8)))
    return res


def measure_hw_ns(inputs, M=64, reps=1):
    import time
    import jax
    from jax.sharding import Mesh, PartitionSpec, NamedSharding
    from jax.experimental.shard_map import shard_map
    from concourse import mybir
    from concourse.bass2jax import _bass_exec_p, install_neuronx_cc_hook, partition_id_tensor

    nc = _get_nc(reps=reps)
    install_neuronx_cc_hook()
    in_maps = _in_maps(inputs)
    partition_name = nc.partition_id_tensor.name if nc.partition_id_tensor else None
    in_names, out_names, out_avals, zero_outs = [], [], [], []
    for alloc in nc.m.functions[0].allocations:
        if not isinstance(alloc, mybir.MemoryLocationSet):
            continue
        name = alloc.memorylocations[0].name
        if alloc.kind == "ExternalInput":
            if name != partition_name:
                in_names.append(name)
        elif alloc.kind == "ExternalOutput":
            out_names.append(name)
            shape = tuple(alloc.tensor_shape)
            dtype = mybir.dt.np(alloc.dtype)
            out_avals.append(jax.core.ShapedArray(shape, dtype))
            zero_outs.append(np.zeros(shape, dtype))
    n_params = len(in_names)
    in_names_all = in_names + out_names
    if partition_name is not None:
        in_names_all.append(partition_name)

    def _body(*args):
        operands = list(args)
        if partition_name is not None:
            operands.append(partition_id_tensor())
        outs = _bass_exec_p.bind(
            *operands, out_avals=tuple(out_avals), in_names=tuple(in_names_all),
            out_names=tuple(out_names), lowering_input_output_aliases=(),
            sim_require_finite=True, sim_require_nnan=True, nc=nc)
        return tuple(outs)

    devices = jax.devices()[:8]
    mesh = Mesh(np.asarray(devices), ("core",))
    spec = PartitionSpec("core")
    fn = jax.jit(
        shard_map(_body, mesh=mesh, in_specs=(spec,) * (n_params + len(out_avals)),
                  out_specs=(spec,) * len(out_avals), check_rep=False),
        keep_unused=True)
    per_core = [[np.asarray(m[name]) for name in in_names] for m in in_maps]
    concat_in = [np.concatenate([per_core[c][i] for c in range(8)], axis=0)
                 for i in range(n_params)]
    concat_zeros = [np.zeros((8 * zz.shape[0], *zz.shape[1:]), zz.dtype) for zz in zero_outs]
    sh = NamedSharding(mesh, spec)
    dev_in = [jax.device_put(a, sh) for a in concat_in]
    dev_zero = [jax.device_put(a, sh) for a in concat_zeros]
    o = fn(*dev_in, *dev_zero)
    jax.block_until_ready(o)
    t0 = time.perf_counter()
    outs = [fn(*dev_in, *dev_zero) for _ in range(M)]
    jax.block_until_ready(outs)
    t1 = time.perf_counter()
    return (t1 - t0) / M * 1e9



# revision 3
# speedup vs baseline: 4.3738x; 4.3738x over previous
import os
import sys

os.environ.setdefault("JAX_PLATFORMS", "")
sys.path.insert(0, "/opt/trn_rl_repo")

import numpy as np
import ml_dtypes

BF16 = ml_dtypes.bfloat16
INV = np.float32(1.0 / np.sqrt(1.0 + 1e-5))
GROUPS = [[0, 1, 2, 3], [4, 5, 6, 7]]
NLOC = 1024
CHUNKS = (0, 512)

_CACHE = {}


def _hilo(a):
    a = np.ascontiguousarray(np.asarray(a, dtype=np.float32))
    hi = a.astype(BF16)
    lo = (a - hi.astype(np.float32)).astype(BF16)
    return hi, lo


def _blockP(a):
    a = np.ascontiguousarray(a)
    R, C = a.shape
    assert R % 128 == 0, (R, C)
    nb = R // 128
    return np.ascontiguousarray(a.reshape(nb, 128, C).transpose(1, 0, 2).reshape(128, nb * C))


# name -> (shape, "bf"/"f32")
SPECS = {
    "xT_hi": ((3, 1024), "bf"), "xT_lo": ((3, 1024), "bf"),
    "c1T_hi": ((3, 64), "bf"), "c1T_lo": ((3, 64), "bf"),
    "c2T_hi": ((64, 128), "bf"), "c2T_lo": ((64, 128), "bf"),
    "c3T_hi": ((128, 256), "bf"), "c3T_lo": ((128, 256), "bf"),
    "p1T_hi": ((128, 512), "bf"), "p1T_lo": ((128, 512), "bf"),
    "p2T_hi": ((128, 512), "bf"), "p2T_lo": ((128, 512), "bf"),
    "fb1": ((128, 1), "f32"), "fb2": ((128, 1), "f32"), "fb3": ((128, 2), "f32"),
    "fbp1": ((128, 2), "f32"), "fbp2": ((128, 2), "f32"),
    "qkT0_hi": ((128, 256), "bf"), "qkT0_lo": ((128, 256), "bf"),
    "qkT1_hi": ((128, 256), "bf"), "qkT1_lo": ((128, 256), "bf"),
    "vwT0_hi": ((128, 512), "bf"), "vwT0_lo": ((128, 512), "bf"),
    "vwT1_hi": ((128, 512), "bf"), "vwT1_lo": ((128, 512), "bf"),
    "twT0_hi": ((128, 512), "bf"), "twT0_lo": ((128, 512), "bf"),
    "twT1_hi": ((128, 512), "bf"), "twT1_lo": ((128, 512), "bf"),
    "vbb0": ((128, 256), "f32"), "vbb1": ((128, 256), "f32"),
    "tbf0": ((128, 2), "f32"), "tbf1": ((128, 2), "f32"),
    "cfT_hi": ((128, 4096), "bf"), "cfT_lo": ((128, 4096), "bf"),
    "cfb": ((128, 4), "f32"),
    "s1fT_hi": ((128, 2048), "bf"), "s1fT_lo": ((128, 2048), "bf"),
    "s1gT_hi": ((128, 2048), "bf"), "s1gT_lo": ((128, 2048), "bf"),
    "s1b": ((128, 4), "f32"),
    "s2T_hi": ((128, 1024), "bf"), "s2T_lo": ((128, 1024), "bf"),
    "s2b": ((128, 2), "f32"),
    "s3T_hi": ((128, 100), "bf"), "s3T_lo": ((128, 100), "bf"),
}


DBG_SPECS = {
    "d_front": ((128, 2048), "f32"),
    "d_xq0": ((128, 1024), "bf"),
    "d_xvt0": ((128, 2048), "bf"),
    "d_xqf": ((128, 4096), "bf"),
    "d_rsg": ((128, 32), "f32"),
    "d_P0": ((128, 1024), "bf"),
    "d_cs": ((1, 1024), "f32"),
    "d_icb": ((128, 1024), "f32"),
    "d_xrh": ((128, 2048), "bf"),
    "d_A": ((128, 2048), "f32"),
    "d_xm1": ((128, 2048), "f32"),
    "d_xm4": ((128, 2048), "f32"),
    "d_faceh": ((128, 4096), "bf"),
    "d_gml": ((128, 4), "f32"),
    "d_gmg": ((128, 4), "f32"),
    "d_gb": ((128, 4), "f32"),
    "d_h2h": ((128, 4096), "bf"),
    "d_h3h": ((128, 2048), "bf"),
}


def _build(debug=False, reps=1):
    from concourse import tile, bacc, mybir

    dt = mybir.dt
    AF = mybir.ActivationFunctionType
    AX = mybir.AxisListType
    ALU = mybir.AluOpType
    bf, f32 = dt.bfloat16, dt.float32

    nc = bacc.Bacc("TRN2", target_bir_lowering=False, debug=False, num_devices=8)

    D = {}
    for nm, (sh, dn) in SPECS.items():
        D[nm] = nc.dram_tensor(nm, list(sh), bf if dn == "bf" else f32, kind="ExternalInput")
    out_d = nc.dram_tensor("out", [50, 1024], f32, kind="ExternalOutput")
    DBG = {}
    if debug:
        for nm, (sh, dn) in DBG_SPECS.items():
            DBG[nm] = nc.dram_tensor(nm, list(sh), bf if dn == "bf" else f32,
                                     kind="ExternalOutput")

    def tap(nm, t):
        if debug and nm in DBG:
            nc.gpsimd.dma_start(DBG[nm][:], t[:])

    with tile.TileContext(nc) as tc:
        with (
            tc.tile_pool(name="pers", bufs=1) as pers,
            tc.tile_pool(name="dramp", bufs=1, space="DRAM") as dramp,
        ):
            def sload(pool, nm):
                sh, dn = SPECS[nm]
                t = pool.tile(list(sh), bf if dn == "bf" else f32, tag=nm, name=nm + "_sb")
                nc.gpsimd.dma_start(t[:], D[nm][:])
                return t

            S = {}
            for nm in ("qkT0_hi", "qkT0_lo", "qkT1_hi", "qkT1_lo",
                       "vwT0_hi", "vwT0_lo", "vwT1_hi", "vwT1_lo",
                       "twT0_hi", "twT0_lo", "twT1_hi", "twT1_lo",
                       "vbb0", "vbb1", "tbf0", "tbf1"):
                S[nm] = sload(pers, nm)

            xm = pers.tile([128, 2048], f32, tag="xm")
            slot_h = [pers.tile([128, 2048], bf, tag=f"sh{s}", name=f"sh{s}") for s in range(5)]
            slot_l = [pers.tile([128, 2048], bf, tag=f"sl{s}", name=f"sl{s}") for s in range(5)]
            ones1 = pers.tile([1, 128], f32, tag="ones1")
            nc.gpsimd.memset(ones1[:], 1.0)

            ag_in = dramp.tile([128, 3072], bf, tag="ag_in")
            ag_out = dramp.tile([512, 3072], bf, tag="ag_out")
            ar_in = dramp.tile([128, 32], f32, tag="ar_in")
            ar_out = dramp.tile([128, 32], f32, tag="ar_out")
            gm_in = dramp.tile([128, 4], f32, tag="gm_in")
            gm_out = dramp.tile([128, 4], f32, tag="gm_out")

            def emit_pipeline():
                # ---------------- front chain ----------------
                with (
                    tc.tile_pool(name="fp", bufs=1) as fp,
                    tc.tile_pool(name="fps", bufs=1, space="PSUM") as fps,
                ):
                    for nm in ("xT_hi", "xT_lo", "c1T_hi", "c1T_lo", "c2T_hi", "c2T_lo",
                               "c3T_hi", "c3T_lo", "p1T_hi", "p1T_lo", "p2T_hi", "p2T_lo",
                               "fb1", "fb2", "fb3", "fbp1", "fbp2"):
                        S[nm] = sload(fp, nm)

                    chain = [
                        ("c1T", 1, 64, "fb1"),
                        ("c2T", 1, 128, "fb2"),
                        ("c3T", 1, 256, "fb3"),
                        ("p1T", 2, 256, "fbp1"),
                        ("p2T", 2, 256, "fbp2"),
                    ]
                    cur_hi, cur_lo = S["xT_hi"], S["xT_lo"]
                    for li, (wn, kb, Cout, bn) in enumerate(chain):
                        whi, wlo, bt = S[wn + "_hi"], S[wn + "_lo"], S[bn]
                        nob = (Cout + 127) // 128
                        P_out = min(Cout, 128)
                        last = li == len(chain) - 1
                        if last:
                            of = xm
                        else:
                            of = fp.tile([P_out, nob * 1024], f32, tag=f"hf{li}", name=f"hf{li}")
                        for ob in range(nob):
                            Mob = min(128, Cout - 128 * ob)
                            for c0 in CHUNKS:
                                ps = fps.tile([Mob, 512], f32, tag="fps_t", bufs=4, name="fps_t")
                                n, tot = 0, kb * 3
                                for kbi in range(kb):
                                    lh = whi[:, kbi * Cout + 128 * ob: kbi * Cout + 128 * ob + Mob]
                                    ll = wlo[:, kbi * Cout + 128 * ob: kbi * Cout + 128 * ob + Mob]
                                    rh = cur_hi[:, kbi * 1024 + c0: kbi * 1024 + c0 + 512]
                                    rl = cur_lo[:, kbi * 1024 + c0: kbi * 1024 + c0 + 512]
                                    for lt, rt in ((lh, rh), (lh, rl), (ll, rh)):
                                        nc.tensor.matmul(ps[:], lt, rt, start=(n == 0), stop=(n == tot - 1))
                                        n += 1
                                nc.scalar.activation(
                                    of[0:Mob, ob * 1024 + c0: ob * 1024 + c0 + 512], ps[:],
                                    AF.Relu, bias=bt[0:Mob, ob: ob + 1],
                                )
                        if last:
                            nh, nl = slot_h[0], slot_l[0]
                        else:
                            nh = fp.tile([P_out, nob * 1024], bf, tag=f"hh{li}", name=f"hh{li}")
                            nl = fp.tile([P_out, nob * 1024], bf, tag=f"hl{li}", name=f"hl{li}")
                        hscr = fp.tile([P_out, nob * 1024], f32, tag="hscr", name="hscr")
                        W = nob * 1024
                        nc.vector.tensor_copy(nh[0:P_out, 0:W], of[0:P_out, 0:W])
                        nc.vector.tensor_copy(hscr[0:P_out, 0:W], nh[0:P_out, 0:W])
                        nc.vector.tensor_sub(nl[0:P_out, 0:W], of[0:P_out, 0:W], hscr[0:P_out, 0:W])
                        cur_hi, cur_lo = nh, nl
                tap("d_front", xm)

                # ---------------- 4 SA layers ----------------
                with tc.tile_pool(name="sap", bufs=1) as sp:
                    for L in range(4):
                        v = 0 if L == 0 else 1
                        ih, il = slot_h[L], slot_l[L]
                        oh, ol = slot_h[L + 1], slot_l[L + 1]
                        qh = S[f"qkT{v}_hi"]
                        vh = S[f"vwT{v}_hi"]
                        th = S[f"twT{v}_hi"]
                        vbb, tbt = S[f"vbb{v}"], S[f"tbf{v}"]

                        xq_loc = sp.tile([128, 1024], bf, tag="xq_loc", name="xq_loc")
                        xvt_loc = sp.tile([128, 2048], bf, tag="xvt_loc", name="xvt_loc")
                        xq_full = sp.tile([128, 4096], bf, tag="xq_full", name="xq_full")
                        xvt_full = sp.tile([128, 8192], bf, tag="xvt_full", name="xvt_full")
                        P_sb = sp.tile([128, 32 * 1024], bf, tag="P_sb", name="P_sb")
                        rs_sb = sp.tile([128, 32], f32, tag="rs_sb", name="rs_sb")
                        rs_g = sp.tile([128, 32], f32, tag="rs_g", name="rs_g")
                        r_f = sp.tile([128, 32], f32, tag="r_f", name="r_f")
                        r_b = sp.tile([128, 32], bf, tag="r_b", name="r_b")
                        A_sb = sp.tile([128, 2048], f32, tag="A_sb", name="A_sb")
                        xr_hi = sp.tile([128, 2048], bf, tag="xr_hi", name="xr_hi")
                        icb = sp.tile([128, 1024], f32, tag="icb", name="icb")
                        cs_row = sp.tile([1, 1024], f32, tag="cs_row", name="cs_row")
                        ic_row = sp.tile([1, 1024], f32, tag="ic_row", name="ic_row")

                        # phase 0: local xq and xv^T (plain bf16)
                        with tc.tile_pool(name="ps0", bufs=1, space="PSUM") as ps0:
                            for c0 in CHUNKS:
                                qps = ps0.tile([128, 512], f32, tag="qps", bufs=2, name="qps")
                                for kbi in range(2):
                                    nc.tensor.matmul(
                                        qps[:], qh[:, kbi * 128: kbi * 128 + 128],
                                        ih[:, kbi * 1024 + c0: kbi * 1024 + c0 + 512],
                                        start=(kbi == 0), stop=(kbi == 1))
                                nc.scalar.activation(xq_loc[:, c0: c0 + 512], qps[:], AF.Copy)
                            for nb in range(8):
                                vps = ps0.tile([128, 256], f32, tag="vps", bufs=2, name="vps")
                                for kbi in range(2):
                                    nc.tensor.matmul(
                                        vps[:], ih[:, kbi * 1024 + nb * 128: kbi * 1024 + nb * 128 + 128],
                                        vh[:, kbi * 256: kbi * 256 + 256],
                                        start=(kbi == 0), stop=(kbi == 1))
                                nc.vector.tensor_add(xvt_loc[:, nb * 256: nb * 256 + 256], vps[:], vbb[:])

                        if L == 0:
                            tap("d_xq0", xq_loc)
                            tap("d_xvt0", xvt_loc)

                        # phase 1: gather xq + xv^T
                        nc.gpsimd.dma_start(ag_in[:, 0:1024], xq_loc[:])
                        nc.gpsimd.dma_start(ag_in[:, 1024:3072], xvt_loc[:])
                        nc.gpsimd.collective_compute(
                            "AllGather", ALU.bypass, replica_groups=GROUPS,
                            ins=[ag_in.opt()], outs=[ag_out.opt()],
                        )
                        for k in range(4):
                            nc.gpsimd.dma_start(xq_full[:, k * 1024: (k + 1) * 1024],
                                                ag_out[k * 128: (k + 1) * 128, 0:1024])
                            nc.gpsimd.dma_start(xvt_full[:, k * 2048: (k + 1) * 2048],
                                                ag_out[k * 128: (k + 1) * 128, 1024:3072])

                        # phase 2: energy + exp(P) + rowsum partials; A = tw@x off critical path
                        with tc.tile_pool(name="ps2", bufs=1, space="PSUM") as ps2:
                            for g in range(32):
                                eps = ps2.tile([128, 1024], f32, tag="eps", bufs=2, name="eps")
                                lhs = xq_full[:, g * 128: (g + 1) * 128]
                                nc.tensor.matmul(eps[:, 0:512], lhs, xq_loc[:, 0:512], start=True, stop=True)
                                nc.tensor.matmul(eps[:, 512:1024], lhs, xq_loc[:, 512:1024], start=True, stop=True)
                                nc.scalar.activation(P_sb[:, g * 1024: (g + 1) * 1024], eps[:],
                                                     AF.Exp, accum_out=rs_sb[:, g: g + 1])
                            for ob in range(2):
                                for c0 in CHUNKS:
                                    aps = ps2.tile([128, 512], f32, tag="aps", bufs=2, name="aps")
                                    for kbi in range(2):
                                        nc.tensor.matmul(
                                            aps[:], th[:, kbi * 256 + 128 * ob: kbi * 256 + 128 * ob + 128],
                                            ih[:, kbi * 1024 + c0: kbi * 1024 + c0 + 512],
                                            start=(kbi == 0), stop=(kbi == 1))
                                    nc.scalar.activation(A_sb[:, ob * 1024 + c0: ob * 1024 + c0 + 512],
                                                         aps[:], AF.Copy)

                        if L == 0:
                            tap("d_xqf", xq_full)
                            tap("d_A", A_sb)
                            if debug:
                                nc.gpsimd.dma_start(DBG["d_P0"][:], P_sb[:, 0:1024])

                        # phase 3: rowsum allreduce, scale, colsum, apply
                        nc.gpsimd.dma_start(ar_in[:], rs_sb[:])
                        nc.gpsimd.collective_compute(
                            "AllReduce", ALU.add, replica_groups=GROUPS,
                            ins=[ar_in.opt()], outs=[ar_out.opt()],
                        )
                        nc.gpsimd.dma_start(rs_g[:], ar_out[:])
                        nc.vector.reciprocal(r_f[:], rs_g[:])
                        nc.vector.tensor_copy(r_b[:], r_f[:])
                        for g in range(32):
                            nc.vector.tensor_scalar_mul(
                                xvt_full[:, g * 256: (g + 1) * 256],
                                xvt_full[:, g * 256: (g + 1) * 256], r_f[:, g: g + 1])

                        with tc.tile_pool(name="ps3", bufs=1, space="PSUM") as ps3:
                            for ci, c0 in enumerate(CHUNKS):
                                csp = ps3.tile([1, 512], f32, tag=f"csp{ci}", name=f"csp{ci}")
                                for g in range(32):
                                    nc.tensor.matmul(csp[:], r_b[:, g: g + 1],
                                                     P_sb[:, g * 1024 + c0: g * 1024 + c0 + 512],
                                                     start=(g == 0), stop=(g == 31))
                                nc.vector.tensor_scalar_add(cs_row[:, c0: c0 + 512], csp[:], 1e-9)
                            nc.vector.reciprocal(ic_row[:], cs_row[:])
                            for c0 in CHUNKS:
                                ibp = ps3.tile([128, 512], f32, tag="ibp", bufs=2, name="ibp")
                                nc.tensor.matmul(ibp[:], ones1[:], ic_row[:, c0: c0 + 512], start=True, stop=True)
                                nc.scalar.activation(icb[:, c0: c0 + 512], ibp[:], AF.Copy)
                            for cb in range(2):
                                for c0 in CHUNKS:
                                    xrp = ps3.tile([128, 512], f32, tag="xrp", bufs=2, name="xrp")
                                    for g in range(32):
                                        nc.tensor.matmul(
                                            xrp[:],
                                            xvt_full[:, g * 256 + cb * 128: g * 256 + cb * 128 + 128],
                                            P_sb[:, g * 1024 + c0: g * 1024 + c0 + 512],
                                            start=(g == 0), stop=(g == 31))
                                    sl = slice(cb * 1024 + c0, cb * 1024 + c0 + 512)
                                    nc.scalar.activation(xr_hi[:, sl], xrp[:], AF.Copy)

                        if L == 0:
                            tap("d_rsg", rs_g)
                            tap("d_cs", cs_row)
                            tap("d_icb", icb)
                            tap("d_xrh", xr_hi)

                        # phase 4: B = tw@xr, y = relu(A - B*icb + tbf), resid add, splits
                        with tc.tile_pool(name="ps4", bufs=1, space="PSUM") as ps4:
                            for ob in range(2):
                                for c0 in CHUNKS:
                                    bps = ps4.tile([128, 512], f32, tag="bps", bufs=2, name="bps")
                                    for kbi in range(2):
                                        nc.tensor.matmul(
                                            bps[:], th[:, kbi * 256 + 128 * ob: kbi * 256 + 128 * ob + 128],
                                            xr_hi[:, kbi * 1024 + c0: kbi * 1024 + c0 + 512],
                                            start=(kbi == 0), stop=(kbi == 1))
                                    osl = slice(ob * 1024 + c0, ob * 1024 + c0 + 512)
                                    tmp = sp.tile([128, 512], f32, tag="scr", bufs=2, name="tmp")
                                    nc.vector.tensor_mul(tmp[:], bps[:], icb[:, c0: c0 + 512])
                                    nc.vector.tensor_sub(A_sb[:, osl], A_sb[:, osl], tmp[:])
                                    yv = sp.tile([128, 512], f32, tag="scr", bufs=2, name="yv")
                                    nc.scalar.activation(yv[:], A_sb[:, osl], AF.Relu, bias=tbt[:, ob: ob + 1])
                                    nc.vector.tensor_add(xm[:, osl], xm[:, osl], yv[:])
                        for c0 in (0, 512, 1024, 1536):
                            sscr = sp.tile([128, 512], f32, tag="scr", bufs=2, name="sscr")
                            nc.vector.tensor_copy(oh[:, c0: c0 + 512], xm[:, c0: c0 + 512])
                            nc.vector.tensor_copy(sscr[:], oh[:, c0: c0 + 512])
                            nc.vector.tensor_sub(ol[:, c0: c0 + 512], xm[:, c0: c0 + 512], sscr[:])
                        if L == 0:
                            tap("d_xm1", xm)
                        if L == 3:
                            tap("d_xm4", xm)

                # ---------------- back end ----------------
                with tc.tile_pool(name="bp", bufs=1) as bp:
                    for nm in ("cfT_hi", "cfT_lo", "cfb", "s1fT_hi", "s1fT_lo",
                               "s1gT_hi", "s1gT_lo", "s1b", "s2T_hi", "s2T_lo",
                               "s2b", "s3T_hi", "s3T_lo"):
                        S[nm] = sload(bp, nm)

                    face_hi = bp.tile([128, 4096], bf, tag="face_hi", name="face_hi")
                    face_lo = bp.tile([128, 4096], bf, tag="face_lo", name="face_lo")
                    gml = bp.tile([128, 4], f32, tag="gml", name="gml")

                    with tc.tile_pool(name="psA", bufs=1, space="PSUM") as psA:
                        for ob in range(4):
                            fscr = bp.tile([128, 1024], f32, tag="fscr", bufs=2, name="fscr")
                            for c0 in CHUNKS:
                                fpt = psA.tile([128, 512], f32, tag="fpsb", bufs=4, name="fpt")
                                n, tot = 0, 24
                                for sk in range(8):
                                    s, cb = 1 + sk // 2, sk % 2
                                    rh = slot_h[s][:, cb * 1024 + c0: cb * 1024 + c0 + 512]
                                    rl = slot_l[s][:, cb * 1024 + c0: cb * 1024 + c0 + 512]
                                    lh = S["cfT_hi"][:, sk * 512 + 128 * ob: sk * 512 + 128 * ob + 128]
                                    ll = S["cfT_lo"][:, sk * 512 + 128 * ob: sk * 512 + 128 * ob + 128]
                                    for lt, rt in ((lh, rh), (lh, rl), (ll, rh)):
                                        nc.tensor.matmul(fpt[:], lt, rt, start=(n == 0), stop=(n == tot - 1))
                                        n += 1
                                nc.scalar.activation(fscr[:, c0: c0 + 512], fpt[:], AF.Prelu,
                                                     bias=S["cfb"][:, ob: ob + 1], alpha=0.2)
                            nc.vector.tensor_reduce(gml[:, ob: ob + 1], fscr[:], axis=AX.X, op=ALU.max)
                            sl = slice(ob * 1024, (ob + 1) * 1024)
                            f2 = bp.tile([128, 1024], f32, tag="f2scr", bufs=2, name="f2")
                            nc.vector.tensor_copy(face_hi[:, sl], fscr[:])
                            nc.vector.tensor_copy(f2[:], face_hi[:, sl])
                            nc.vector.tensor_sub(face_lo[:, sl], fscr[:], f2[:])

                        nc.gpsimd.dma_start(gm_in[:], gml[:])
                        nc.gpsimd.collective_compute(
                            "AllReduce", ALU.max, replica_groups=GROUPS,
                            ins=[gm_in.opt()], outs=[gm_out.opt()],
                        )
                        gmg = bp.tile([128, 4], f32, tag="gmg", name="gmg")
                        nc.gpsimd.dma_start(gmg[:], gm_out[:])
                        gmh = bp.tile([128, 4], bf, tag="gmh", name="gmh")
                        gmlo = bp.tile([128, 4], bf, tag="gmlo", name="gmlo")
                        gms = bp.tile([128, 4], f32, tag="gms", name="gms")
                        nc.vector.tensor_copy(gmh[:], gmg[:])
                        nc.vector.tensor_copy(gms[:], gmh[:])
                        nc.vector.tensor_sub(gmlo[:], gmg[:], gms[:])

                        gb = bp.tile([128, 4], f32, tag="gb", name="gb")
                        for ob in range(4):
                            gvp = psA.tile([128, 1], f32, tag="gvp", bufs=2, name="gvp")
                            n, tot = 0, 12
                            for kbi in range(4):
                                lh = S["s1gT_hi"][:, kbi * 512 + 128 * ob: kbi * 512 + 128 * ob + 128]
                                ll = S["s1gT_lo"][:, kbi * 512 + 128 * ob: kbi * 512 + 128 * ob + 128]
                                rh = gmh[:, kbi: kbi + 1]
                                rl = gmlo[:, kbi: kbi + 1]
                                for lt, rt in ((lh, rh), (lh, rl), (ll, rh)):
                                    nc.tensor.matmul(gvp[:], lt, rt, start=(n == 0), stop=(n == tot - 1))
                                    n += 1
                            nc.vector.tensor_add(gb[:, ob: ob + 1], gvp[:], S["s1b"][:, ob: ob + 1])
                        tap("d_faceh", face_hi)
                        tap("d_gml", gml)
                        tap("d_gmg", gmg)
                        tap("d_gb", gb)

                    h2h = bp.tile([128, 4096], bf, tag="h2h", name="h2h")
                    h2l = bp.tile([128, 4096], bf, tag="h2l", name="h2l")
                    h3h = bp.tile([128, 2048], bf, tag="h3h", name="h3h")
                    h3l = bp.tile([128, 2048], bf, tag="h3l", name="h3l")
                    outsb = bp.tile([50, 1024], f32, tag="outsb", name="outsb")

                    with tc.tile_pool(name="psB", bufs=1, space="PSUM") as psB:
                        for ob in range(4):
                            uscr = bp.tile([128, 1024], f32, tag="fscr", bufs=2, name="uscr")
                            for c0 in CHUNKS:
                                sp1 = psB.tile([128, 512], f32, tag="sp1", bufs=2, name="sp1")
                                n, tot = 0, 12
                                for kbi in range(4):
                                    lh = S["s1fT_hi"][:, kbi * 512 + 128 * ob: kbi * 512 + 128 * ob + 128]
                                    ll = S["s1fT_lo"][:, kbi * 512 + 128 * ob: kbi * 512 + 128 * ob + 128]
                                    rh = face_hi[:, kbi * 1024 + c0: kbi * 1024 + c0 + 512]
                                    rl = face_lo[:, kbi * 1024 + c0: kbi * 1024 + c0 + 512]
                                    for lt, rt in ((lh, rh), (lh, rl), (ll, rh)):
                                        nc.tensor.matmul(sp1[:], lt, rt, start=(n == 0), stop=(n == tot - 1))
                                        n += 1
                                nc.scalar.activation(uscr[:, c0: c0 + 512], sp1[:], AF.Prelu,
                                                     bias=gb[:, ob: ob + 1], alpha=0.2)
                            sl = slice(ob * 1024, (ob + 1) * 1024)
                            f2 = bp.tile([128, 1024], f32, tag="f2scr", bufs=2, name="f2b")
                            nc.vector.tensor_copy(h2h[:, sl], uscr[:])
                            nc.vector.tensor_copy(f2[:], h2h[:, sl])
                            nc.vector.tensor_sub(h2l[:, sl], uscr[:], f2[:])

                        for ob in range(2):
                            u2 = bp.tile([128, 1024], f32, tag="fscr", bufs=2, name="u2")
                            for c0 in CHUNKS:
                                sp2 = psB.tile([128, 512], f32, tag="sp2", bufs=2, name="sp2")
                                n, tot = 0, 12
                                for kbi in range(4):
                                    lh = S["s2T_hi"][:, kbi * 256 + 128 * ob: kbi * 256 + 128 * ob + 128]
                                    ll = S["s2T_lo"][:, kbi * 256 + 128 * ob: kbi * 256 + 128 * ob + 128]
                                    rh = h2h[:, kbi * 1024 + c0: kbi * 1024 + c0 + 512]
                                    rl = h2l[:, kbi * 1024 + c0: kbi * 1024 + c0 + 512]
                                    for lt, rt in ((lh, rh), (lh, rl), (ll, rh)):
                                        nc.tensor.matmul(sp2[:], lt, rt, start=(n == 0), stop=(n == tot - 1))
                                        n += 1
                                nc.scalar.activation(u2[:, c0: c0 + 512], sp2[:], AF.Prelu,
                                                     bias=S["s2b"][:, ob: ob + 1], alpha=0.2)
                            sl = slice(ob * 1024, (ob + 1) * 1024)
                            f2 = bp.tile([128, 1024], f32, tag="f2scr", bufs=2, name="f2c")
                            nc.vector.tensor_copy(h3h[:, sl], u2[:])
                            nc.vector.tensor_copy(f2[:], h3h[:, sl])
                            nc.vector.tensor_sub(h3l[:, sl], u2[:], f2[:])

                        for c0 in CHUNKS:
                            sp3 = psB.tile([50, 512], f32, tag="sp3", bufs=2, name="sp3")
                            n, tot = 0, 6
                            for kbi in range(2):
                                lh = S["s3T_hi"][:, kbi * 50: kbi * 50 + 50]
                                ll = S["s3T_lo"][:, kbi * 50: kbi * 50 + 50]
                                rh = h3h[:, kbi * 1024 + c0: kbi * 1024 + c0 + 512]
                                rl = h3l[:, kbi * 1024 + c0: kbi * 1024 + c0 + 512]
                                for lt, rt in ((lh, rh), (lh, rl), (ll, rh)):
                                    nc.tensor.matmul(sp3[:], lt, rt, start=(n == 0), stop=(n == tot - 1))
                                    n += 1
                            nc.scalar.activation(outsb[:, c0: c0 + 512], sp3[:], AF.Copy)

                    tap("d_h2h", h2h)
                    tap("d_h3h", h3h)
                    nc.gpsimd.dma_start(out_d[:], outsb[:])

            for _ in range(reps):
                emit_pipeline()

    nc.compile()
    return nc


def _prep_shared(inputs):
    g = lambda k: np.asarray(inputs[k], np.float32)
    out = {}

    def fold(wn, gn, bn):
        return g(wn) * (INV * g(gn))[:, None], g(bn)

    def emit(nm, wf):
        wT = np.ascontiguousarray(wf.T)
        if wT.shape[0] > 128:
            wT = _blockP(wT)
        hi, lo = _hilo(wT)
        out[nm + "_hi"], out[nm + "_lo"] = hi, lo

    w1, b1 = fold("conv1_w", "bn1_g", "bn1_b")
    w2, b2 = fold("conv2_w", "bn2_g", "bn2_b")
    w3, b3 = fold("conv3_w", "bn3_g", "bn3_b")
    wp1, bp1 = fold("pt1_w", "pt1_g", "pt1_b")
    wp2, bp2 = fold("pt2_w", "pt2_g", "pt2_b")
    emit("c1T", w1)
    emit("c2T", w2)
    emit("c3T", w3)
    emit("p1T", wp1)
    emit("p2T", wp2)
    fb1 = np.zeros((128, 1), np.float32)
    fb1[:64, 0] = b1
    out["fb1"] = fb1
    out["fb2"] = np.ascontiguousarray(b2[:, None])
    out["fb3"] = _blockP(b3[:, None]).astype(np.float32)
    out["fbp1"] = _blockP(bp1[:, None]).astype(np.float32)
    out["fbp2"] = _blockP(bp2[:, None]).astype(np.float32)

    for v, p in ((0, "sa1"), (1, "sa2")):
        emit(f"qkT{v}", g(p + "_qk"))
        emit(f"vwT{v}", g(p + "_vw"))
        sg, sb2 = g(p + "_g"), g(p + "_b")
        twf = g(p + "_tw") * (INV * sg)[:, None]
        emit(f"twT{v}", twf)
        out[f"vbb{v}"] = np.ascontiguousarray(
            np.broadcast_to(g(p + "_vb")[None, :], (128, 256))).astype(np.float32)
        tbfv = g(p + "_tb") * (INV * sg) + sb2
        out[f"tbf{v}"] = _blockP(tbfv[:, None]).astype(np.float32)

    cfw, cfb_ = fold("cf_w", "cf_g", "cf_b")
    emit("cfT", cfw)
    out["cfb"] = _blockP(cfb_[:, None]).astype(np.float32)
    s1w, s1b_ = fold("s1_w", "s1_g", "s1_b")
    emit("s1fT", s1w[:, :512])
    emit("s1gT", s1w[:, 512:])
    out["s1b"] = _blockP(s1b_[:, None]).astype(np.float32)
    s2w, s2b_ = fold("s2_w", "s2_g", "s2_b")
    emit("s2T", s2w)
    out["s2b"] = _blockP(s2b_[:, None]).astype(np.float32)
    emit("s3T", g("s3_w"))

    for nm, (sh, dn) in SPECS.items():
        if nm.startswith("xT"):
            continue
        a = out[nm]
        assert tuple(a.shape) == sh, (nm, a.shape, sh)
        assert (a.dtype == BF16) == (dn == "bf"), (nm, a.dtype)
    return out


def _get_nc(debug=False, reps=1):
    key = ("nc_dbg" if debug else "nc") + str(reps)
    if key not in _CACHE:
        _CACHE[key] = _build(debug, reps)
    return _CACHE[key]


def _in_maps(inputs):
    base = _prep_shared(inputs)
    x = np.asarray(inputs["x"], np.float32)
    maps = []
    for c in range(8):
        b, j = c // 4, c % 4
        xT = np.ascontiguousarray(x[b, 1024 * j: 1024 * (j + 1), :].T)
        hi, lo = _hilo(xT)
        m = dict(base)
        m["xT_hi"], m["xT_lo"] = hi, lo
        maps.append(m)
    return maps


def _assemble(res):
    full = np.empty((2, 4096, 50), np.float32)
    for c in range(8):
        b, j = c // 4, c % 4
        full[b, 1024 * j: 1024 * (j + 1), :] = np.asarray(res.results[c]["out"], np.float32).T
    return full


def _run_preput(nc, in_maps):
    """Execute the prebuilt Bass module on 8 cores via one sharded PJRT call,
    with all inputs pre-placed on device so every core launches together
    (otherwise per-core H2D transfer skew is absorbed into the first
    collective wait on the early cores)."""
    import jax
    from jax.sharding import Mesh, PartitionSpec, NamedSharding
    from jax.experimental.shard_map import shard_map
    from concourse import mybir
    from concourse.bass2jax import (
        _bass_exec_p, install_neuronx_cc_hook, partition_id_tensor)

    install_neuronx_cc_hook()
    partition_name = nc.partition_id_tensor.name if nc.partition_id_tensor else None
    in_names, out_names, out_avals, zero_outs = [], [], [], []
    for alloc in nc.m.functions[0].allocations:
        if not isinstance(alloc, mybir.MemoryLocationSet):
            continue
        name = alloc.memorylocations[0].name
        if alloc.kind == "ExternalInput":
            if name != partition_name:
                in_names.append(name)
        elif alloc.kind == "ExternalOutput":
            out_names.append(name)
            shape = tuple(alloc.tensor_shape)
            dtype = mybir.dt.np(alloc.dtype)
            out_avals.append(jax.core.ShapedArray(shape, dtype))
            zero_outs.append(np.zeros(shape, dtype))
    n_params = len(in_names)
    in_names_all = in_names + out_names
    if partition_name is not None:
        in_names_all.append(partition_name)

    def _body(*args):
        operands = list(args)
        if partition_name is not None:
            operands.append(partition_id_tensor())
        outs = _bass_exec_p.bind(
            *operands, out_avals=tuple(out_avals), in_names=tuple(in_names_all),
            out_names=tuple(out_names), lowering_input_output_aliases=(),
            sim_require_finite=True, sim_require_nnan=True, nc=nc)
        return tuple(outs)

    devices = jax.devices()[:8]
    mesh = Mesh(np.asarray(devices), ("core",))
    spec = PartitionSpec("core")
    fn = jax.jit(
        shard_map(_body, mesh=mesh, in_specs=(spec,) * (n_params + len(out_avals)),
                  out_specs=(spec,) * len(out_avals), check_rep=False),
        keep_unused=True)
    per_core = [[np.asarray(m[name]) for name in in_names] for m in in_maps]
    concat_in = [np.concatenate([per_core[c][i] for c in range(8)], axis=0)
                 for i in range(n_params)]
    concat_zeros = [np.zeros((8 * zz.shape[0], *zz.shape[1:]), zz.dtype)
                    for zz in zero_outs]
    sh = NamedSharding(mesh, spec)
    dev_in = [jax.device_put(a, sh) for a in concat_in]
    dev_zero = [jax.device_put(a, sh) for a in concat_zeros]
    jax.block_until_ready(dev_in)
    jax.block_until_ready(dev_zero)
    # Compile before the timed/traced execution so tracing+XLA compile
    # don't sit between device placement and launch.
    fn_c = fn.lower(*dev_in, *dev_zero).compile()
    out_arrs = fn_c(*dev_in, *dev_zero)
    jax.block_until_ready(out_arrs)
    return [
        {name: np.asarray(out_arrs[i]).reshape(8, *out_avals[i].shape)[c]
         for i, name in enumerate(out_names)}
        for c in range(8)
    ]


def kernel(**inputs):
    nc = _get_nc()
    results = _run_preput(nc, _in_maps(inputs))

    class _R:
        pass

    res = _R()
    res.results = results
    return _assemble(res)


def run_traced(inputs, trace_cores=None):
    from concourse.bass_utils import run_bass_kernel_spmd
    nc = _get_nc()
    res = run_bass_kernel_spmd(
        nc, _in_maps(inputs), core_ids=list(range(8)),
        trace=True, trace_cores=trace_cores or [0],
    )
    return _assemble(res), res


def run_debug(inputs):
    from concourse.bass_utils import run_bass_kernel_spmd
    nc = _get_nc(debug=True)
    res = run_bass_kernel_spmd(nc, _in_maps(inputs), core_ids=list(range(8)))
    return res


def measure_hw_ns(inputs, M=64, reps=1):
    import time
    import jax
    from jax.sharding import Mesh, PartitionSpec, NamedSharding
    from jax.experimental.shard_map import shard_map
    from concourse import mybir
    from concourse.bass2jax import _bass_exec_p, install_neuronx_cc_hook, partition_id_tensor

    nc = _get_nc(reps=reps)
    install_neuronx_cc_hook()
    in_maps = _in_maps(inputs)
    partition_name = nc.partition_id_tensor.name if nc.partition_id_tensor else None
    in_names, out_names, out_avals, zero_outs = [], [], [], []
    for alloc in nc.m.functions[0].allocations:
        if not isinstance(alloc, mybir.MemoryLocationSet):
            continue
        name = alloc.memorylocations[0].name
        if alloc.kind == "ExternalInput":
            if name != partition_name:
                in_names.append(name)
        elif alloc.kind == "ExternalOutput":
            out_names.append(name)
            shape = tuple(alloc.tensor_shape)
            dtype = mybir.dt.np(alloc.dtype)
            out_avals.append(jax.core.ShapedArray(shape, dtype))
            zero_outs.append(np.zeros(shape, dtype))
    n_params = len(in_names)
    in_names_all = in_names + out_names
    if partition_name is not None:
        in_names_all.append(partition_name)

    def _body(*args):
        operands = list(args)
        if partition_name is not None:
            operands.append(partition_id_tensor())
        outs = _bass_exec_p.bind(
            *operands, out_avals=tuple(out_avals), in_names=tuple(in_names_all),
            out_names=tuple(out_names), lowering_input_output_aliases=(),
            sim_require_finite=True, sim_require_nnan=True, nc=nc)
        return tuple(outs)

    devices = jax.devices()[:8]
    mesh = Mesh(np.asarray(devices), ("core",))
    spec = PartitionSpec("core")
    fn = jax.jit(
        shard_map(_body, mesh=mesh, in_specs=(spec,) * (n_params + len(out_avals)),
                  out_specs=(spec,) * len(out_avals), check_rep=False),
        keep_unused=True)
    per_core = [[np.asarray(m[name]) for name in in_names] for m in in_maps]
    concat_in = [np.concatenate([per_core[c][i] for c in range(8)], axis=0)
                 for i in range(n_params)]
    concat_zeros = [np.zeros((8 * zz.shape[0], *zz.shape[1:]), zz.dtype) for zz in zero_outs]
    sh = NamedSharding(mesh, spec)
    dev_in = [jax.device_put(a, sh) for a in concat_in]
    dev_zero = [jax.device_put(a, sh) for a in concat_zeros]
    o = fn(*dev_in, *dev_zero)
    jax.block_until_ready(o)
    t0 = time.perf_counter()
    outs = [fn(*dev_in, *dev_zero) for _ in range(M)]
    jax.block_until_ready(outs)
    t1 = time.perf_counter()
    return (t1 - t0) / M * 1e9



# revision 8
# speedup vs baseline: 5.5757x; 1.2748x over previous
import os
import sys

os.environ.setdefault("JAX_PLATFORMS", "")
sys.path.insert(0, "/opt/trn_rl_repo")

import numpy as np
import ml_dtypes

BF16 = ml_dtypes.bfloat16
INV = np.float32(1.0 / np.sqrt(1.0 + 1e-5))
GROUPS = [[0, 1, 2, 3], [4, 5, 6, 7]]
CHUNKS = (0, 512)

_CACHE = {}


def _blockP(a):
    a = np.ascontiguousarray(a)
    R, C = a.shape
    assert R % 128 == 0, (R, C)
    nb = R // 128
    return np.ascontiguousarray(a.reshape(nb, 128, C).transpose(1, 0, 2).reshape(128, nb * C))


# name -> (shape, "bf"/"f32")
SPECS = {
    "xT_hi": ((3, 1024), "bf"), "xT_lo": ((3, 1024), "bf"),
    "c1T": ((3, 64), "bf"),
    "c2T": ((64, 128), "bf"),
    "c3T": ((128, 256), "bf"),
    "p1T": ((128, 512), "bf"),
    "p2T": ((128, 512), "bf"),
    "fb1": ((128, 1), "f32"), "fb2": ((128, 1), "f32"), "fb3": ((128, 2), "f32"),
    "fbp1": ((128, 2), "f32"), "fbp2": ((128, 2), "f32"),
    "qkT0": ((128, 256), "bf"), "qkT1": ((128, 256), "bf"),
    "vwT0": ((128, 512), "bf"), "vwT1": ((128, 512), "bf"),
    "twT0": ((128, 512), "bf"), "twT1": ((128, 512), "bf"),
    "vbb0": ((128, 256), "f32"), "vbb1": ((128, 256), "f32"),
    "tbf0": ((128, 2), "f32"), "tbf1": ((128, 2), "f32"),
    "cfT": ((128, 4096), "bf"), "cfb": ((128, 4), "f32"),
    "s1fT": ((128, 2048), "bf"), "s1gT": ((128, 2048), "bf"),
    "s1b": ((128, 4), "f32"),
    "s2T": ((128, 1024), "bf"), "s2b": ((128, 2), "f32"),
    "s3T": ((128, 100), "bf"),
}


def _build(reps=1):
    from concourse import tile, bacc, mybir

    dt = mybir.dt
    AF = mybir.ActivationFunctionType
    AX = mybir.AxisListType
    ALU = mybir.AluOpType
    bf, f32 = dt.bfloat16, dt.float32

    nc = bacc.Bacc("TRN2", target_bir_lowering=False, debug=False, num_devices=8)

    D = {}
    for nm, (sh, dn) in SPECS.items():
        D[nm] = nc.dram_tensor(nm, list(sh), bf if dn == "bf" else f32, kind="ExternalInput")
    out_d = nc.dram_tensor("out", [50, 1024], f32, kind="ExternalOutput")

    with tile.TileContext(nc) as tc:
        with (
            tc.tile_pool(name="pers", bufs=1) as pers,
            tc.tile_pool(name="dramp", bufs=1, space="DRAM") as dramp,
        ):
            def sload(pool, nm):
                sh, dn = SPECS[nm]
                t = pool.tile(list(sh), bf if dn == "bf" else f32, tag=nm, name=nm + "_sb")
                nc.gpsimd.dma_start(t[:], D[nm][:])
                return t

            S = {}
            for nm in ("qkT0", "qkT1", "vwT0", "vwT1", "twT0", "twT1",
                       "vbb0", "vbb1", "tbf0", "tbf1"):
                S[nm] = sload(pers, nm)

            xm = pers.tile([128, 2048], f32, tag="xm")
            # bf16 layer outputs: slot[0]=front, slot[L+1]=SA layer L output
            slot = [pers.tile([128, 2048], bf, tag=f"s{s}", name=f"s{s}") for s in range(5)]
            ones1 = pers.tile([1, 128], f32, tag="ones1")
            nc.gpsimd.memset(ones1[:], 1.0)

            agq_in = dramp.tile([128, 1024], bf, tag="agq_in")
            agq_out = dramp.tile([512, 1024], bf, tag="agq_out")
            agv_in = dramp.tile([128, 2048], bf, tag="agv_in")
            agv_out = dramp.tile([512, 2048], bf, tag="agv_out")
            ar1_in = dramp.tile([128, 16], f32, tag="ar1_in")
            ar1_out = dramp.tile([128, 16], f32, tag="ar1_out")
            ar2_in = dramp.tile([128, 16], f32, tag="ar2_in")
            ar2_out = dramp.tile([128, 16], f32, tag="ar2_out")
            gm_in = dramp.tile([128, 4], f32, tag="gm_in")
            gm_out = dramp.tile([128, 4], f32, tag="gm_out")

            def emit_pipeline():
                # ---------------- front chain ----------------
                with (
                    tc.tile_pool(name="fp", bufs=1) as fp,
                    tc.tile_pool(name="fps", bufs=1, space="PSUM") as fps,
                ):
                    for nm in ("xT_hi", "xT_lo", "c1T", "c2T", "c3T", "p1T", "p2T",
                               "fb1", "fb2", "fb3", "fbp1", "fbp2"):
                        S[nm] = sload(fp, nm)

                    chain = [
                        ("c1T", 1, 64, "fb1"),
                        ("c2T", 1, 128, "fb2"),
                        ("c3T", 1, 256, "fb3"),
                        ("p1T", 2, 256, "fbp1"),
                        ("p2T", 2, 256, "fbp2"),
                    ]
                    cur = S["xT_hi"]
                    cur_lo = S["xT_lo"]
                    for li, (wn, kb, Cout, bn) in enumerate(chain):
                        wt, bt = S[wn], S[bn]
                        nob = (Cout + 127) // 128
                        P_out = min(Cout, 128)
                        last = li == len(chain) - 1
                        if last:
                            nh = slot[0]
                        else:
                            nh = fp.tile([P_out, nob * 1024], bf, tag=f"hh{li}", name=f"hh{li}")
                        for ob in range(nob):
                            Mob = min(128, Cout - 128 * ob)
                            for c0 in CHUNKS:
                                ps = fps.tile([Mob, 512], f32, tag="fps_t", bufs=4, name="fps_t")
                                tot = kb + (1 if li == 0 else 0)
                                n = 0
                                for kbi in range(kb):
                                    lh = wt[:, kbi * Cout + 128 * ob: kbi * Cout + 128 * ob + Mob]
                                    rh = cur[:, kbi * 1024 + c0: kbi * 1024 + c0 + 512]
                                    nc.tensor.matmul(ps[:], lh, rh, start=(n == 0), stop=(n == tot - 1))
                                    n += 1
                                    if li == 0:
                                        rl = cur_lo[:, kbi * 1024 + c0: kbi * 1024 + c0 + 512]
                                        nc.tensor.matmul(ps[:], lh, rl, start=False, stop=(n == tot - 1))
                                        n += 1
                                osl = slice(ob * 1024 + c0, ob * 1024 + c0 + 512)
                                if last:
                                    nc.scalar.activation(xm[0:Mob, osl], ps[:], AF.Relu,
                                                         bias=bt[0:Mob, ob: ob + 1])
                                    nc.vector.tensor_copy(nh[0:Mob, osl], xm[0:Mob, osl])
                                else:
                                    nc.scalar.activation(nh[0:Mob, osl], ps[:], AF.Relu,
                                                         bias=bt[0:Mob, ob: ob + 1])
                        cur = nh
                        cur_lo = None

                # ---------------- 4 SA layers ----------------
                with tc.tile_pool(name="sap", bufs=1) as sp:
                    for L in range(4):
                        v = 0 if L == 0 else 1
                        ih = slot[L]
                        oh = slot[L + 1]
                        qh = S[f"qkT{v}"]
                        vh = S[f"vwT{v}"]
                        th = S[f"twT{v}"]
                        vbb, tbt = S[f"vbb{v}"], S[f"tbf{v}"]

                        xq_loc = sp.tile([128, 1024], bf, tag="xq_loc", name="xq_loc")
                        xvt_loc = sp.tile([128, 2048], bf, tag="xvt_loc", name="xvt_loc")
                        xq_full = sp.tile([128, 4096], bf, tag="xq_full", name="xq_full")
                        xvt_full = sp.tile([128, 8192], bf, tag="xvt_full", name="xvt_full")
                        P_sb = sp.tile([128, 32 * 1024], bf, tag="P_sb", name="P_sb")
                        rs_sb = sp.tile([128, 32], f32, tag="rs_sb", name="rs_sb")
                        rs_g = sp.tile([128, 32], f32, tag="rs_g", name="rs_g")
                        r_f = sp.tile([128, 32], f32, tag="r_f", name="r_f")
                        r_b = sp.tile([128, 32], bf, tag="r_b", name="r_b")
                        A_sb = sp.tile([128, 2048], f32, tag="A_sb", name="A_sb")
                        xr_hi = sp.tile([128, 2048], bf, tag="xr_hi", name="xr_hi")
                        icb = sp.tile([128, 1024], f32, tag="icb", name="icb")
                        cs_row = sp.tile([1, 1024], f32, tag="cs_row", name="cs_row")
                        ic_row = sp.tile([1, 1024], f32, tag="ic_row", name="ic_row")

                        # phase 0: local xq and xv^T; AG triggers asap
                        with tc.tile_pool(name="ps0", bufs=1, space="PSUM") as ps0:
                            for c0 in CHUNKS:
                                qps = ps0.tile([128, 512], f32, tag="qps", bufs=2, name="qps")
                                for kbi in range(2):
                                    nc.tensor.matmul(
                                        qps[:], qh[:, kbi * 128: kbi * 128 + 128],
                                        ih[:, kbi * 1024 + c0: kbi * 1024 + c0 + 512],
                                        start=(kbi == 0), stop=(kbi == 1))
                                nc.vector.tensor_copy(xq_loc[:, c0: c0 + 512], qps[:])
                            nc.gpsimd.dma_start(agq_in[:], xq_loc[:])
                            nc.gpsimd.collective_compute(
                                "AllGather", ALU.bypass, replica_groups=GROUPS,
                                ins=[agq_in.opt()], outs=[agq_out.opt()],
                            )
                            for nb in range(8):
                                vps = ps0.tile([128, 256], f32, tag="vps", bufs=2, name="vps")
                                for kbi in range(2):
                                    nc.tensor.matmul(
                                        vps[:], ih[:, kbi * 1024 + nb * 128: kbi * 1024 + nb * 128 + 128],
                                        vh[:, kbi * 256: kbi * 256 + 256],
                                        start=(kbi == 0), stop=(kbi == 1))
                                nc.vector.tensor_add(xvt_loc[:, nb * 256: nb * 256 + 256], vps[:], vbb[:])
                            nc.gpsimd.dma_start(agv_in[:], xvt_loc[:])
                            nc.gpsimd.collective_compute(
                                "AllGather", ALU.bypass, replica_groups=GROUPS,
                                ins=[agv_in.opt()], outs=[agv_out.opt()],
                            )
                            # A = tw @ x while the AllGathers run
                            for ob in range(2):
                                for c0 in CHUNKS:
                                    aps = ps0.tile([128, 512], f32, tag="aps", bufs=2, name="aps")
                                    for kbi in range(2):
                                        nc.tensor.matmul(
                                            aps[:], th[:, kbi * 256 + 128 * ob: kbi * 256 + 128 * ob + 128],
                                            ih[:, kbi * 1024 + c0: kbi * 1024 + c0 + 512],
                                            start=(kbi == 0), stop=(kbi == 1))
                                    nc.vector.tensor_copy(A_sb[:, ob * 1024 + c0: ob * 1024 + c0 + 512],
                                                          aps[:])
                        for k in range(4):
                            nc.gpsimd.dma_start(xq_full[:, k * 1024: (k + 1) * 1024],
                                                agq_out[k * 128: (k + 1) * 128, :])
                        for k in range(4):
                            nc.gpsimd.dma_start(xvt_full[:, k * 2048: (k + 1) * 2048],
                                                agv_out[k * 128: (k + 1) * 128, :])

                        # phase 1: energy/exp pipeline (PSUM: eps 2x2=4 banks)
                        # phase 2: x_r chains overlap the exp tail (xrp 4 banks)
                        with tc.tile_pool(name="ps1", bufs=1, space="PSUM") as ps1:
                                def emit_energy(g):
                                    eps = ps1.tile([128, 1024], f32, tag="eps", bufs=2, name="eps")
                                    lhs = xq_full[:, g * 128: (g + 1) * 128]
                                    nc.tensor.matmul(eps[:, 0:512], lhs, xq_loc[:, 0:512],
                                                     start=True, stop=True)
                                    nc.tensor.matmul(eps[:, 512:1024], lhs, xq_loc[:, 512:1024],
                                                     start=True, stop=True)
                                    nc.scalar.activation(P_sb[:, g * 1024: (g + 1) * 1024], eps[:],
                                                         AF.Exp, accum_out=rs_sb[:, g: g + 1])

                                xrp = [ps1.tile([128, 512], f32, tag=f"xrp{i}", name=f"xrp{i}")
                                       for i in range(4)]

                                def emit_xr(g, lastg):
                                    nc.vector.tensor_scalar_mul(
                                        xvt_full[:, g * 256: (g + 1) * 256],
                                        xvt_full[:, g * 256: (g + 1) * 256], r_f[:, g: g + 1])
                                    for cb in range(2):
                                        for ci, c0 in enumerate(CHUNKS):
                                            nc.tensor.matmul(
                                                xrp[cb * 2 + ci][:],
                                                xvt_full[:, g * 256 + cb * 128: g * 256 + cb * 128 + 128],
                                                P_sb[:, g * 1024 + c0: g * 1024 + c0 + 512],
                                                start=(g == 0), stop=lastg)

                                for g in range(16):
                                    emit_energy(g)
                                nc.gpsimd.dma_start(ar1_in[:], rs_sb[:, 0:16])
                                nc.gpsimd.collective_compute(
                                    "AllReduce", ALU.add, replica_groups=GROUPS,
                                    ins=[ar1_in.opt()], outs=[ar1_out.opt()],
                                )
                                for g in range(16, 32):
                                    emit_energy(g)
                                nc.gpsimd.dma_start(ar2_in[:], rs_sb[:, 16:32])
                                nc.gpsimd.collective_compute(
                                    "AllReduce", ALU.add, replica_groups=GROUPS,
                                    ins=[ar2_in.opt()], outs=[ar2_out.opt()],
                                )

                                nc.gpsimd.dma_start(rs_g[:, 0:16], ar1_out[:])
                                nc.vector.reciprocal(r_f[:, 0:16], rs_g[:, 0:16])
                                nc.vector.tensor_copy(r_b[:, 0:16], r_f[:, 0:16])
                                for g in range(16):
                                    emit_xr(g, False)

                                nc.gpsimd.dma_start(rs_g[:, 16:32], ar2_out[:])
                                nc.vector.reciprocal(r_f[:, 16:32], rs_g[:, 16:32])
                                nc.vector.tensor_copy(r_b[:, 16:32], r_f[:, 16:32])
                                for g in range(16, 32):
                                    emit_xr(g, g == 31)

                                for cb in range(2):
                                    for ci, c0 in enumerate(CHUNKS):
                                        nc.vector.tensor_copy(
                                            xr_hi[:, cb * 1024 + c0: cb * 1024 + c0 + 512],
                                            xrp[cb * 2 + ci][:])

                        # colsum chains + icb + B use the banks freed above
                        with tc.tile_pool(name="ps2", bufs=1, space="PSUM") as ps2:
                                csp = [ps2.tile([1, 512], f32, tag=f"csp{ci}", name=f"csp{ci}")
                                       for ci in range(2)]
                                for g in range(32):
                                    for ci, c0 in enumerate(CHUNKS):
                                        nc.tensor.matmul(
                                            csp[ci][:], r_b[:, g: g + 1],
                                            P_sb[:, g * 1024 + c0: g * 1024 + c0 + 512],
                                            start=(g == 0), stop=(g == 31))
                                for ci, c0 in enumerate(CHUNKS):
                                    nc.vector.tensor_scalar_add(cs_row[:, c0: c0 + 512], csp[ci][:], 1e-9)
                                nc.vector.reciprocal(ic_row[:], cs_row[:])
                                for c0 in CHUNKS:
                                    ibp = ps2.tile([128, 512], f32, tag="ibp", bufs=2, name="ibp")
                                    nc.tensor.matmul(ibp[:], ones1[:], ic_row[:, c0: c0 + 512],
                                                     start=True, stop=True)
                                    nc.vector.tensor_copy(icb[:, c0: c0 + 512], ibp[:])

                                # phase 3: B = tw@xr, y = relu(A - B*icb + tbf), resid add
                                for ob in range(2):
                                    for c0 in CHUNKS:
                                        bps = ps2.tile([128, 512], f32, tag="bps", bufs=2, name="bps")
                                        for kbi in range(2):
                                            nc.tensor.matmul(
                                                bps[:], th[:, kbi * 256 + 128 * ob: kbi * 256 + 128 * ob + 128],
                                                xr_hi[:, kbi * 1024 + c0: kbi * 1024 + c0 + 512],
                                                start=(kbi == 0), stop=(kbi == 1))
                                        osl = slice(ob * 1024 + c0, ob * 1024 + c0 + 512)
                                        tmp = sp.tile([128, 512], f32, tag="scr", bufs=2, name="tmp")
                                        nc.vector.tensor_mul(tmp[:], bps[:], icb[:, c0: c0 + 512])
                                        nc.vector.tensor_sub(A_sb[:, osl], A_sb[:, osl], tmp[:])
                                        yv = sp.tile([128, 512], f32, tag="scr", bufs=2, name="yv")
                                        nc.scalar.activation(yv[:], A_sb[:, osl], AF.Relu,
                                                             bias=tbt[:, ob: ob + 1])
                                        nc.vector.tensor_add(xm[:, osl], xm[:, osl], yv[:])
                                        nc.vector.tensor_copy(oh[:, osl], xm[:, osl])

                # ---------------- back end ----------------
                with tc.tile_pool(name="bp", bufs=1) as bp:
                    for nm in ("cfT", "cfb", "s1fT", "s1gT", "s1b", "s2T", "s2b", "s3T"):
                        S[nm] = sload(bp, nm)

                    face_h = bp.tile([128, 4096], bf, tag="face_h", name="face_h")
                    gml = bp.tile([128, 4], f32, tag="gml", name="gml")

                    with tc.tile_pool(name="psA", bufs=1, space="PSUM") as psA:
                        for ob in range(4):
                            for c0 in CHUNKS:
                                fpt = psA.tile([128, 512], f32, tag="fpsb", bufs=4, name="fpt")
                                n, tot = 0, 8
                                for sk in range(8):
                                    s, cb = 1 + sk // 2, sk % 2
                                    rh = slot[s][:, cb * 1024 + c0: cb * 1024 + c0 + 512]
                                    lh = S["cfT"][:, sk * 512 + 128 * ob: sk * 512 + 128 * ob + 128]
                                    nc.tensor.matmul(fpt[:], lh, rh, start=(n == 0), stop=(n == tot - 1))
                                    n += 1
                                nc.scalar.activation(face_h[:, ob * 1024 + c0: ob * 1024 + c0 + 512],
                                                     fpt[:], AF.Prelu,
                                                     bias=S["cfb"][:, ob: ob + 1], alpha=0.2)
                            sl = slice(ob * 1024, (ob + 1) * 1024)
                            nc.vector.tensor_reduce(gml[:, ob: ob + 1], face_h[:, sl], axis=AX.X, op=ALU.max)

                        nc.gpsimd.dma_start(gm_in[:], gml[:])
                        nc.gpsimd.collective_compute(
                            "AllReduce", ALU.max, replica_groups=GROUPS,
                            ins=[gm_in.opt()], outs=[gm_out.opt()],
                        )

                        # s1 matmuls on the face part run during the AllReduce;
                        # pre-bias results staged in SBUF until gb arrives
                        h2h = bp.tile([128, 4096], bf, tag="h2h", name="h2h")
                        uscr = bp.tile([128, 4096], f32, tag="uscr", name="uscr")
                        for ob in range(4):
                            for c0 in CHUNKS:
                                sp1 = psA.tile([128, 512], f32, tag="sp1", bufs=2, name="sp1")
                                for kbi in range(4):
                                    lh = S["s1fT"][:, kbi * 512 + 128 * ob: kbi * 512 + 128 * ob + 128]
                                    rh = face_h[:, kbi * 1024 + c0: kbi * 1024 + c0 + 512]
                                    nc.tensor.matmul(sp1[:], lh, rh, start=(kbi == 0), stop=(kbi == 3))
                                nc.vector.tensor_copy(uscr[:, ob * 1024 + c0: ob * 1024 + c0 + 512],
                                                      sp1[:])

                        gmg = bp.tile([128, 4], f32, tag="gmg", name="gmg")
                        nc.gpsimd.dma_start(gmg[:], gm_out[:])
                        gmh = bp.tile([128, 4], bf, tag="gmh", name="gmh")
                        nc.vector.tensor_copy(gmh[:], gmg[:])

                        gb = bp.tile([128, 4], f32, tag="gb", name="gb")
                        for ob in range(4):
                            gvp = psA.tile([128, 1], f32, tag="gvp", bufs=2, name="gvp")
                            for kbi in range(4):
                                lh = S["s1gT"][:, kbi * 512 + 128 * ob: kbi * 512 + 128 * ob + 128]
                                rh = gmh[:, kbi: kbi + 1]
                                nc.tensor.matmul(gvp[:], lh, rh, start=(kbi == 0), stop=(kbi == 3))
                            nc.vector.tensor_add(gb[:, ob: ob + 1], gvp[:], S["s1b"][:, ob: ob + 1])

                        for ob in range(4):
                            for c0 in CHUNKS:
                                nc.scalar.activation(h2h[:, ob * 1024 + c0: ob * 1024 + c0 + 512],
                                                     uscr[:, ob * 1024 + c0: ob * 1024 + c0 + 512],
                                                     AF.Prelu, bias=gb[:, ob: ob + 1], alpha=0.2)

                    h3h = bp.tile([128, 2048], bf, tag="h3h", name="h3h")
                    outsb = bp.tile([50, 1024], f32, tag="outsb", name="outsb")

                    with tc.tile_pool(name="psB", bufs=1, space="PSUM") as psB:
                        for ob in range(2):
                            for c0 in CHUNKS:
                                sp2 = psB.tile([128, 512], f32, tag="sp2", bufs=2, name="sp2")
                                for kbi in range(4):
                                    lh = S["s2T"][:, kbi * 256 + 128 * ob: kbi * 256 + 128 * ob + 128]
                                    rh = h2h[:, kbi * 1024 + c0: kbi * 1024 + c0 + 512]
                                    nc.tensor.matmul(sp2[:], lh, rh, start=(kbi == 0), stop=(kbi == 3))
                                nc.scalar.activation(h3h[:, ob * 1024 + c0: ob * 1024 + c0 + 512],
                                                     sp2[:], AF.Prelu,
                                                     bias=S["s2b"][:, ob: ob + 1], alpha=0.2)

                        for c0 in CHUNKS:
                            sp3 = psB.tile([50, 512], f32, tag="sp3", bufs=2, name="sp3")
                            n = 0
                            for kbi in range(2):
                                lh = S["s3T"][:, kbi * 50: kbi * 50 + 50]
                                rh = h3h[:, kbi * 1024 + c0: kbi * 1024 + c0 + 512]
                                nc.tensor.matmul(sp3[:], lh, rh, start=(n == 0), stop=(n == 1))
                                n += 1
                            nc.vector.tensor_copy(outsb[:, c0: c0 + 512], sp3[:])

                    nc.gpsimd.dma_start(out_d[:], outsb[:])

            for _ in range(reps):
                emit_pipeline()

    nc.compile()
    return nc


def _prep_shared(inputs):
    g = lambda k: np.asarray(inputs[k], np.float32)
    out = {}

    def fold(wn, gn, bn):
        return g(wn) * (INV * g(gn))[:, None], g(bn)

    def emit(nm, wf):
        wT = np.ascontiguousarray(wf.T)
        if wT.shape[0] > 128:
            wT = _blockP(wT)
        out[nm] = np.ascontiguousarray(wT.astype(BF16))

    w1, b1 = fold("conv1_w", "bn1_g", "bn1_b")
    w2, b2 = fold("conv2_w", "bn2_g", "bn2_b")
    w3, b3 = fold("conv3_w", "bn3_g", "bn3_b")
    wp1, bp1 = fold("pt1_w", "pt1_g", "pt1_b")
    wp2, bp2 = fold("pt2_w", "pt2_g", "pt2_b")
    emit("c1T", w1)
    emit("c2T", w2)
    emit("c3T", w3)
    emit("p1T", wp1)
    emit("p2T", wp2)
    fb1 = np.zeros((128, 1), np.float32)
    fb1[:64, 0] = b1
    out["fb1"] = fb1
    out["fb2"] = np.ascontiguousarray(b2[:, None])
    out["fb3"] = _blockP(b3[:, None]).astype(np.float32)
    out["fbp1"] = _blockP(bp1[:, None]).astype(np.float32)
    out["fbp2"] = _blockP(bp2[:, None]).astype(np.float32)

    for v, p in ((0, "sa1"), (1, "sa2")):
        emit(f"qkT{v}", g(p + "_qk"))
        emit(f"vwT{v}", g(p + "_vw"))
        sg, sb2 = g(p + "_g"), g(p + "_b")
        twf = g(p + "_tw") * (INV * sg)[:, None]
        emit(f"twT{v}", twf)
        out[f"vbb{v}"] = np.ascontiguousarray(
            np.broadcast_to(g(p + "_vb")[None, :], (128, 256))).astype(np.float32)
        tbfv = g(p + "_tb") * (INV * sg) + sb2
        out[f"tbf{v}"] = _blockP(tbfv[:, None]).astype(np.float32)

    cfw, cfb_ = fold("cf_w", "cf_g", "cf_b")
    emit("cfT", cfw)
    out["cfb"] = _blockP(cfb_[:, None]).astype(np.float32)
    s1w, s1b_ = fold("s1_w", "s1_g", "s1_b")
    emit("s1fT", s1w[:, :512])
    emit("s1gT", s1w[:, 512:])
    out["s1b"] = _blockP(s1b_[:, None]).astype(np.float32)
    s2w, s2b_ = fold("s2_w", "s2_g", "s2_b")
    emit("s2T", s2w)
    out["s2b"] = _blockP(s2b_[:, None]).astype(np.float32)
    emit("s3T", g("s3_w"))

    for nm, (sh, dn) in SPECS.items():
        if nm.startswith("xT"):
            continue
        a = out[nm]
        assert tuple(a.shape) == sh, (nm, a.shape, sh)
        assert (a.dtype == BF16) == (dn == "bf"), (nm, a.dtype)
    return out


def _get_nc(reps=1):
    key = "nc" + str(reps)
    if key not in _CACHE:
        _CACHE[key] = _build(reps)
    return _CACHE[key]


def _hilo(a):
    a = np.ascontiguousarray(np.asarray(a, dtype=np.float32))
    hi = a.astype(BF16)
    lo = (a - hi.astype(np.float32)).astype(BF16)
    return hi, lo


def _in_maps(inputs):
    base = _prep_shared(inputs)
    x = np.asarray(inputs["x"], np.float32)
    maps = []
    for c in range(8):
        b, j = c // 4, c % 4
        xT = np.ascontiguousarray(x[b, 1024 * j: 1024 * (j + 1), :].T)
        hi, lo = _hilo(xT)
        m = dict(base)
        m["xT_hi"], m["xT_lo"] = hi, lo
        maps.append(m)
    return maps


def _assemble(results):
    full = np.empty((2, 4096, 50), np.float32)
    for c in range(8):
        b, j = c // 4, c % 4
        full[b, 1024 * j: 1024 * (j + 1), :] = np.asarray(results[c]["out"], np.float32).T
    return full


def _run_preput(nc, in_maps):
    """Execute the prebuilt Bass module on 8 cores via one sharded PJRT call,
    with all inputs pre-placed on device so every core launches together
    (otherwise per-core H2D transfer skew is absorbed into the first
    collective wait on the early cores)."""
    import jax
    from jax.sharding import Mesh, PartitionSpec, NamedSharding
    from jax.experimental.shard_map import shard_map
    from concourse import mybir
    from concourse.bass2jax import (
        _bass_exec_p, install_neuronx_cc_hook, partition_id_tensor)

    install_neuronx_cc_hook()
    partition_name = nc.partition_id_tensor.name if nc.partition_id_tensor else None
    in_names, out_names, out_avals, zero_outs = [], [], [], []
    for alloc in nc.m.functions[0].allocations:
        if not isinstance(alloc, mybir.MemoryLocationSet):
            continue
        name = alloc.memorylocations[0].name
        if alloc.kind == "ExternalInput":
            if name != partition_name:
                in_names.append(name)
        elif alloc.kind == "ExternalOutput":
            out_names.append(name)
            shape = tuple(alloc.tensor_shape)
            dtype = mybir.dt.np(alloc.dtype)
            out_avals.append(jax.core.ShapedArray(shape, dtype))
            zero_outs.append(np.zeros(shape, dtype))
    n_params = len(in_names)
    in_names_all = in_names + out_names
    if partition_name is not None:
        in_names_all.append(partition_name)

    def _body(*args):
        operands = list(args)
        if partition_name is not None:
            operands.append(partition_id_tensor())
        outs = _bass_exec_p.bind(
            *operands, out_avals=tuple(out_avals), in_names=tuple(in_names_all),
            out_names=tuple(out_names), lowering_input_output_aliases=(),
            sim_require_finite=True, sim_require_nnan=True, nc=nc)
        return tuple(outs)

    devices = jax.devices()[:8]
    mesh = Mesh(np.asarray(devices), ("core",))
    spec = PartitionSpec("core")
    fn = jax.jit(
        shard_map(_body, mesh=mesh, in_specs=(spec,) * (n_params + len(out_avals)),
                  out_specs=(spec,) * len(out_avals), check_rep=False),
        keep_unused=True)
    per_core = [[np.asarray(m[name]) for name in in_names] for m in in_maps]
    concat_in = [np.concatenate([per_core[c][i] for c in range(8)], axis=0)
                 for i in range(n_params)]
    concat_zeros = [np.zeros((8 * zz.shape[0], *zz.shape[1:]), zz.dtype)
                    for zz in zero_outs]
    sh = NamedSharding(mesh, spec)
    dev_in = [jax.device_put(a, sh) for a in concat_in]
    dev_zero = [jax.device_put(a, sh) for a in concat_zeros]
    jax.block_until_ready(dev_in)
    jax.block_until_ready(dev_zero)
    # Compile before the timed/traced execution so tracing+XLA compile
    # don't sit between device placement and launch.
    fn_c = fn.lower(*dev_in, *dev_zero).compile()
    out_arrs = fn_c(*dev_in, *dev_zero)
    jax.block_until_ready(out_arrs)
    return [
        {name: np.asarray(out_arrs[i]).reshape(8, *out_avals[i].shape)[c]
         for i, name in enumerate(out_names)}
        for c in range(8)
    ]


def kernel(**inputs):
    nc = _get_nc()
    results = _run_preput(nc, _in_maps(inputs))
    return _assemble(results)


def measure_hw_ns(inputs, M=64, reps=1):
    import time
    import jax
    from jax.sharding import Mesh, PartitionSpec, NamedSharding
    from jax.experimental.shard_map import shard_map
    from concourse import mybir
    from concourse.bass2jax import _bass_exec_p, install_neuronx_cc_hook, partition_id_tensor

    nc = _get_nc(reps=reps)
    install_neuronx_cc_hook()
    in_maps = _in_maps(inputs)
    partition_name = nc.partition_id_tensor.name if nc.partition_id_tensor else None
    in_names, out_names, out_avals, zero_outs = [], [], [], []
    for alloc in nc.m.functions[0].allocations:
        if not isinstance(alloc, mybir.MemoryLocationSet):
            continue
        name = alloc.memorylocations[0].name
        if alloc.kind == "ExternalInput":
            if name != partition_name:
                in_names.append(name)
        elif alloc.kind == "ExternalOutput":
            out_names.append(name)
            shape = tuple(alloc.tensor_shape)
            dtype = mybir.dt.np(alloc.dtype)
            out_avals.append(jax.core.ShapedArray(shape, dtype))
            zero_outs.append(np.zeros(shape, dtype))
    n_params = len(in_names)
    in_names_all = in_names + out_names
    if partition_name is not None:
        in_names_all.append(partition_name)

    def _body(*args):
        operands = list(args)
        if partition_name is not None:
            operands.append(partition_id_tensor())
        outs = _bass_exec_p.bind(
            *operands, out_avals=tuple(out_avals), in_names=tuple(in_names_all),
            out_names=tuple(out_names), lowering_input_output_aliases=(),
            sim_require_finite=True, sim_require_nnan=True, nc=nc)
        return tuple(outs)

    devices = jax.devices()[:8]
    mesh = Mesh(np.asarray(devices), ("core",))
    spec = PartitionSpec("core")
    fn = jax.jit(
        shard_map(_body, mesh=mesh, in_specs=(spec,) * (n_params + len(out_avals)),
                  out_specs=(spec,) * len(out_avals), check_rep=False),
        keep_unused=True)
    per_core = [[np.asarray(m[name]) for name in in_names] for m in in_maps]
    concat_in = [np.concatenate([per_core[c][i] for c in range(8)], axis=0)
                 for i in range(n_params)]
    concat_zeros = [np.zeros((8 * zz.shape[0], *zz.shape[1:]), zz.dtype) for zz in zero_outs]
    sh = NamedSharding(mesh, spec)
    dev_in = [jax.device_put(a, sh) for a in concat_in]
    dev_zero = [jax.device_put(a, sh) for a in concat_zeros]
    o = fn(*dev_in, *dev_zero)
    jax.block_until_ready(o)
    t0 = time.perf_counter()
    outs = [fn(*dev_in, *dev_zero) for _ in range(M)]
    jax.block_until_ready(outs)
    t1 = time.perf_counter()
    return (t1 - t0) / M * 1e9
